# revision 2
# baseline (speedup 1.0000x reference)
"""Trainium2 Bass kernel for the focal-modulation dense_cnn problem (v5).

Math (per reference):
  fx = conv1x1(x, f_w, f_b);  q, gates = fx[:, :C], fx[:, C:]
  ctx = sum_l x_list[l] * gates[:, l]
  mod = conv1x1(ctx, h_w, h_b)
  y   = conv1x1(q * mod, proj_w, proj_b)
  out = layernorm_c(y) * ln_w + ln_b + x

Strategy (data-parallel, 2 batches/core, 8 cores; F=512-pixel tiles):
  * fx runs channel-major ([97,2,F] moving, fp16) with the 3 gate channels
    folded into the stationary (M=99 piece) -- 4 matmuls.
  * Gates are transposed on the PE ([3,128] -> [128,3] per 128-px group) so
    the gating multiply becomes DVE tensor_scalar with per-partition fp32
    scalars (4x perf mode on fp16 SBUF operands): 12 ts + 2 adds per tile.
  * ctx is transposed back to channel-major on the PE (8 tiny transposes)
    and copied PSUM->SBUF; mod is 4 channel-major matmuls; xo=(mod+hb)*q is
    scalar_tensor_tensor on Pool.
  * proj uses xo as the *stationary* operand ([96,128] chunks) with the
    weight matrix moving, so y lands pixel-major [128pix,192ch] in PSUM
    (8 matmuls of 192 rows + 4 bias-broadcast matmuls) -- the LayerNorm
    tail then needs no partition reductions: var = Act Square+accum_out,
    istd is a per-partition scalar, apply = stt((y*istd) + x0') with the
    residual and ln bias folded into a host-precomputed pixel-major x0'.
  * All activation tensors fp16 (DMA bytes halved); matmuls fp16 (1 cyc/row).
"""

import sys

sys.path.insert(0, "/opt/trn_rl_repo")

import numpy as np

import bass_rust
import concourse.bass as bass
import concourse.mybir as mybir
import concourse.tile as tile
from concourse.bass_utils import run_bass_kernel_spmd
from concourse.vector_clock import ScopedClock

# ---------------------------------------------------------------------------
# Workaround: this walrus build accepts only one sem wait per instruction
# ("Too many sync wait commands"). (1) chain the Tile tail drain's waits;
# (2) post-pass that moves excess waits onto NoOps inserted just before the
# offending instruction on the same engine.


def _patched_drain_and_barrier(self, tick_clock, wait_clock):
    nc = self.nc
    drain_inst = nc.sync.drain()
    wait_clock.add_sem_waits(
        drain_inst.ins, ScopedClock({None: tick_clock.global_clock})
    )
    si = drain_inst.ins.sync_info
    if si is not None and len(si.on_wait) > 1:
        waits = list(si.on_wait)
        drain_inst.ins.sync_info = bass_rust.SyncInfo(
            on_wait=waits[:1], on_update=list(si.on_update)
        )
        for w in waits[1:]:
            d2 = nc.sync.drain()
            d2.ins.sync_info = bass_rust.SyncInfo(on_wait=[w], on_update=[])
    nc.all_engine_barrier()
    assert self.sems is not None
    popped = nc._tile_sem_poison_stack.pop()
    assert popped is self._sem_poison
    nc.clear_and_free_semaphores(list(self.sems.allocated().values()))
    nc.all_engine_barrier()


tile.TileContext._drain_and_barrier = _patched_drain_and_barrier

_WAIT_LIMIT = 1


def _split_excess_waits(nc):
    k = 0
    for f in nc.m.functions:
        for b in f.blocks:
            il = b.instructions
            new = []
            for inst in il:
                si = inst.sync_info
                if si is not None and len(si.on_wait) > _WAIT_LIMIT:
                    waits = list(si.on_wait)
                    excess, keep = waits[:-_WAIT_LIMIT], waits[-_WAIT_LIMIT:]
                    for w in excess:
                        nop = mybir.InstNoOp(name=f"wsplit-{k}",
                                             engine=inst.engine)
                        nop.sync_info = bass_rust.SyncInfo(on_wait=[w],
                                                           on_update=[])
                        new.append(nop)
                        k += 1
                    inst.sync_info = bass_rust.SyncInfo(
                        on_wait=keep, on_update=list(si.on_update))
                new.append(inst)
            il[:] = new
    return k
# ---------------------------------------------------------------------------

FP32 = mybir.dt.float32
F16 = mybir.dt.float16
AF = mybir.ActivationFunctionType
OP = mybir.AluOpType

NCORES = 8
N_FULL, C, H, W, L = 16, 192, 128, 128, 3
HW = H * W
NS = N_FULL // NCORES          # batch per core
MAC = 1024                     # pixels per DMA macro-tile
F = 512                        # pixels per inner tile
NMAC = HW // MAC
NF = MAC // F
NCH = F // 256                 # 256-px chunks per F tile (=2)
EPS = 1e-6
RSC = float(1.0 / np.sqrt(C))  # variance via Square(in*RSC) accumulation

_prog_cache = {}


def _ileave(base):
    """[P, 256] contiguous AP -> [P, (par 2)(idx 128)] interleaved view.

    Free enumeration (par, idx) maps position par*128+idx to element
    par + 2*idx, so channel-major pixel order within each 256-px chunk
    matches the pixel-major packing (partition p <-> pixel 2p+par).
    """
    return bass_rust.AP(tensor=base.tensor, offset=base.offset,
                        ap=[base.ap[0], [1, 2], [2, 128]])


def _build_program(general_lnw, general_pb):
    nc = bass.Bass(trn_type="TRN2")

    d_x = nc.dram_tensor("x", [NS, 194, HW], F16, kind="ExternalInput")
    d_xl = [nc.dram_tensor(f"xl{l}", [NS, HW, C], F16, kind="ExternalInput")
            for l in range(L)]
    d_x0 = nc.dram_tensor("x0p", [NS, HW, C], F16, kind="ExternalInput")
    d_fw = [nc.dram_tensor(f"fw{j}", [97, 195], F16, kind="ExternalInput")
            for j in range(2)]
    d_hw = [nc.dram_tensor(f"hw{j}", [96, 192], F16, kind="ExternalInput")
            for j in range(2)]
    d_pj = [nc.dram_tensor(f"pj{j}", [96, 192], F16, kind="ExternalInput")
            for j in range(2)]
    d_pbt = nc.dram_tensor("pbt", [1, 192], F16, kind="ExternalInput")
    d_hb = nc.dram_tensor("hb", [96, 2], FP32, kind="ExternalInput")
    d_id16 = nc.dram_tensor("id16", [128, 128], F16, kind="ExternalInput")
    d_id32 = nc.dram_tensor("id32", [99, 3], F16, kind="ExternalInput")
    d_one1 = nc.dram_tensor("one1", [1, 128], F16, kind="ExternalInput")
    d_eps = nc.dram_tensor("epsc", [128, 1], FP32, kind="ExternalInput")
    if general_lnw:
        d_lwp = nc.dram_tensor("lwp", [128, 192], F16, kind="ExternalInput")
    d_out = nc.dram_tensor("out", [NS, HW, C], F16, kind="ExternalOutput")

    # x: [NS, 194, HW] -> [NS][97, 2, HW]; aug channel a = 97*j + p
    vx = d_x[:, :, :].rearrange("n (j p) w -> n p j w", j=2)
    # pixel-major tensors: pixel = 1024*m + 256*cc + 2*p + par
    vxl = [t[:, :, :].rearrange("n (m cc p par) c -> n m p cc (par c)",
                                m=NMAC, cc=4, p=128, par=2) for t in d_xl]
    vx0 = d_x0[:, :, :].rearrange("n (m cc p par) c -> n m p cc (par c)",
                                  m=NMAC, cc=4, p=128, par=2)
    vout = d_out[:, :, :].rearrange("n (m cc p par) c -> n m p cc (par c)",
                                    m=NMAC, cc=4, p=128, par=2)

    from contextlib import ExitStack
    with tile.TileContext(nc) as tc, ExitStack() as ctx:
        sing = ctx.enter_context(tc.tile_pool(name="sing", bufs=1))
        inp = ctx.enter_context(tc.tile_pool(name="inp", bufs=4))
        qgp = ctx.enter_context(tc.tile_pool(name="qgp", bufs=4))
        gat = ctx.enter_context(tc.tile_pool(name="gat", bufs=2))
        csb = ctx.enter_context(tc.tile_pool(name="csb", bufs=2))
        xop = ctx.enter_context(tc.tile_pool(name="xop", bufs=3))
        tlp = ctx.enter_context(tc.tile_pool(name="tlp", bufs=2))
        zp = ctx.enter_context(tc.tile_pool(name="zp", bufs=3))
        pfx = ctx.enter_context(tc.tile_pool(name="pfx", bufs=1, space="PSUM"))
        pgp = ctx.enter_context(tc.tile_pool(name="pgp", bufs=1, space="PSUM"))
        pcx = ctx.enter_context(tc.tile_pool(name="pcx", bufs=1, space="PSUM"))
        pmd = ctx.enter_context(tc.tile_pool(name="pmd", bufs=1, space="PSUM"))
        ppj = ctx.enter_context(tc.tile_pool(name="ppj", bufs=2, space="PSUM"))

        # ---- load constants / weights (once) ----
        def loadw(dram, shape, dt, tag):
            t = sing.tile(shape, dt, tag=tag, name=tag)
            nc.sync.dma_start(out=t,
                              in_=dram[tuple(slice(0, s) for s in shape)])
            return t

        fw = [loadw(d_fw[j], [97, 195], F16, f"fw{j}") for j in range(2)]
        hwt = [loadw(d_hw[j], [96, 192], F16, f"hw{j}") for j in range(2)]
        pjc = [loadw(d_pj[j], [96, 192], F16, f"pj{j}") for j in range(2)]
        pbt = loadw(d_pbt, [1, 192], F16, "pbt")
        hb = loadw(d_hb, [96, 2], FP32, "hb")
        id16 = loadw(d_id16, [128, 128], F16, "id16")
        id32 = loadw(d_id32, [99, 3], F16, "id32")
        one1 = loadw(d_one1, [1, 128], F16, "one1")
        eps_t = loadw(d_eps, [128, 1], FP32, "epsc")
        lwp = loadw(d_lwp, [128, 192], F16, "lwp") if general_lnw else None

        mm = nc.tensor.matmul
        tiles = [(n, im, fi) for n in range(NS) for im in range(NMAC)
                 for fi in range(NF)]
        T = len(tiles)
        macs = {}
        st = {}

        def load_macro(n, im):
            o0 = im * MAC
            x_t = inp.tile([97, 2, MAC], F16, tag="x", name="x_t")
            nc.sync.dma_start(out=x_t, in_=vx[n, :, :, o0:o0 + MAC])
            xl_t = []
            for l in range(L):
                t = inp.tile([128, 4, 2 * C], F16, tag=f"xl{l}",
                             name=f"xl{l}")
                nc.sync.dma_start(out=t, in_=vxl[l][n, im])
                xl_t.append(t)
            x0_t = inp.tile([128, 4, 2 * C], F16, tag="x0", name="x0_t")
            nc.sync.dma_start(out=x0_t, in_=vx0[n, im])
            macs[(n, im)] = (x_t, xl_t, x0_t)

        # ---- stage S0: DMA + fx matmuls + q/gates PSUM->SBUF copies ----
        def _xmov(x_t, j, fi):
            """fx moving operand: [97, F] with free order (cc, par, idx)
            matching the pixel-major group packing."""
            base = x_t[:, j, fi * F:(fi + 1) * F]
            return bass_rust.AP(tensor=base.tensor, offset=base.offset,
                                ap=[base.ap[0], [256, 2], [1, 2], [2, 128]])

        def s0(i):
            n, im, fi = tiles[i]
            if (n, im) not in macs:
                load_macro(n, im)
            x_t, xl_t, x0_t = macs[(n, im)]
            fx = pfx.tile([99, 2, F], FP32, tag="fx", name="fx")
            for j in range(2):
                mm(fx[:, 0, :], fw[j][:, 0:99], _xmov(x_t, j, fi),
                   start=(j == 0), stop=(j == 1))
            for j in range(2):
                mm(fx[0:96, 1, :], fw[j][:, 99:195], _xmov(x_t, j, fi),
                   start=(j == 0), stop=(j == 1))
            # q + gates to SBUF in one copy (fp32: gate transposes/ts
            # scalars need fp32)
            qg = qgp.tile([99, 2, F], F16, tag="qg", name="qg")
            nc.scalar.activation(qg, fx, AF.Copy)
            st[i] = {"n": n, "im": im, "fi": fi,
                     "xl_t": xl_t, "x0_t": x0_t, "qg": qg}

        # ---- stage S1: gate transposes + gating (pixel-major) ----
        def s1(i):
            d = st[i]
            fi = d["fi"]
            qg0 = d["qg"][:, 0, :]
            xl_t = d["xl_t"]
            gts = gat.tile([3, F], F16, tag="gts", name="gts")
            nc.sync.dma_start(out=gts, in_=qg0[96:99, :])
            gps = pgp.tile([128, 4, 4], F16, tag="gps", name="gps")
            for cp in range(4):
                nc.tensor.transpose(gps[:, cp, 0:3],
                                    gts[:, cp * 128:(cp + 1) * 128],
                                    id32[0:3, :])
            gpsf = gat.tile([128, 4, 4], FP32, tag="gpsf", name="gpsf")
            nc.vector.tensor_scalar(gpsf, gps, 1.0, None, OP.mult)
            t0 = gat.tile([128, 4, C], F16, tag="t0", name="t0")
            t1 = gat.tile([128, 4, C], F16, tag="t1", name="t1")
            t2 = gat.tile([128, 4, C], F16, tag="t2", name="t2")
            tl = [t0, t1, t2]
            for l in range(L):
                for cc in range(NCH):
                    for par in range(2):
                        cp = 2 * cc + par
                        eng = nc.vector if l < 2 else nc.gpsimd
                        eng.tensor_scalar(
                            tl[l][:, cp, :],
                            xl_t[l][:, 2 * fi + cc,
                                    par * C:(par + 1) * C],
                            gpsf[:, cp, l:l + 1], None, OP.mult)
            cxa = gat.tile([128, 4, C], F16, tag="cxa", name="cxa")
            nc.vector.tensor_tensor(cxa, t0, t1, OP.add)
            cxb = gat.tile([128, 4, C], F16, tag="cxb", name="cxb")
            nc.vector.tensor_tensor(cxb, cxa, t2, OP.add)
            d["ctx_px"] = cxb

        # ---- stage S2: ctx transposes back to channel-major + copy ----
        def s2(i):
            d = st[i]
            cxb = d["ctx_px"]
            ccm = pcx.tile([96, 2, F], F16, tag="ccm", name="ccm")
            for j in range(2):
                for cp in range(4):
                    nc.tensor.transpose(
                        ccm[:, j, cp * 128:(cp + 1) * 128],
                        cxb[:, cp, 96 * j:96 * (j + 1)], id16)
            csb_t = csb.tile([96, 2, F], F16, tag="csb", name="csb_t")
            nc.vector.tensor_scalar(csb_t, ccm, 1.0, None, OP.mult)
            d["ctx_sb"] = csb_t

        # ---- stage S3: mod matmuls + xo = (mod+hb)*q on Pool ----
        def s3(i):
            d = st[i]
            ctx_sb = d["ctx_sb"]
            mod = pmd.tile([96, 2, F], FP32, tag="mod", name="mod")
            for m in range(2):
                for j in range(2):
                    mm(mod[:, m, :], hwt[j][:, 96 * m:96 * (m + 1)],
                       ctx_sb[:, j, :], start=(j == 0), stop=(j == 1))
            msb = xop.tile([96, 2, F], F16, tag="msb", name="msb")
            for m in range(2):
                nc.scalar.activation(msb[:, m, :], mod[:, m, :],
                                     AF.Identity, bias=hb[:, m:m + 1])
            xo = xop.tile([96, 2, F], F16, tag="xo", name="xo")
            nc.vector.tensor_tensor(xo, msb, d["qg"][0:96, :, :], OP.mult)
            d["xo"] = xo

        # ---- stage S4a/S4b: proj (pixel-major out) + LN tail, one half
        # (= one 256-px chunk pair... 2 of the 4 groups) per step so the
        # pj PSUM pool double-buffers within the bank budget ----
        zmacs = {}

        def s4h(i, h):
            d = st[i]
            xo = d["xo"]
            x0_t = d["x0_t"]
            n, im, fi = d["n"], d["im"], d["fi"]
            if (n, im) not in zmacs:
                zmacs[(n, im)] = zp.tile([128, 4, 2 * C], F16, tag="zmac",
                                         name="zmac")
            zmac = zmacs[(n, im)]
            pjh = ppj.tile([128, 2, 192], FP32, tag="pjh", name="pjh")
            for ci in range(2):
                cp = 2 * h + ci
                out = pjh[:, ci, :]
                for j in range(2):
                    mm(out, xo[:, j, cp * 128:(cp + 1) * 128], pjc[j],
                       start=(j == 0), stop=(j == 1 and not general_pb))
                if general_pb:
                    mm(out, one1, pbt, start=False, stop=True)
            var2 = tlp.tile([128, 2], FP32, tag="var2", name="var2")
            sqs = tlp.tile([128, 2, 192], F16, tag="sqs", name="sqs")
            nc.scalar.activation(sqs, pjh, AF.Square, scale=RSC)
            for ci in range(2):
                nc.vector.tensor_reduce(var2[:, ci:ci + 1], sqs[:, ci, :],
                                        mybir.AxisListType.X, OP.add)
            lnv = tlp.tile([128, 2], FP32, tag="lnv", name="lnv")
            nc.scalar.activation(lnv, var2, AF.Ln, bias=eps_t)
            istd = tlp.tile([128, 2], FP32, tag="istd", name="istd")
            nc.scalar.activation(istd, lnv, AF.Exp, scale=-0.5)
            z1 = tlp.tile([128, 2 * C], F16, tag="z1", name="z1")
            for ci in range(2):
                nc.vector.tensor_scalar(z1[:, ci * C:(ci + 1) * C],
                                        pjh[:, ci, :],
                                        istd[:, ci:ci + 1], None, OP.mult)
                if general_lnw:
                    nc.vector.tensor_tensor(z1[:, ci * C:(ci + 1) * C],
                                            z1[:, ci * C:(ci + 1) * C],
                                            lwp, OP.mult)
            nc.vector.tensor_tensor(zmac[:, 2 * fi + h, :], z1,
                                    x0_t[:, 2 * fi + h, :], OP.add)
            if h == 1:
                st.pop(i)
                if fi == NF - 1:
                    nc.sync.dma_start(out=vout[n, im], in_=zmac)
                    del zmacs[(n, im)]
                    del macs[(n, im)]

        # ---- software pipeline: 6 slots ----
        import os
        order = os.environ.get("K5_ORDER", "134502")
        slots = {"0": (s0, 0), "1": (s1, 1), "2": (s2, 2), "3": (s3, 3),
                 "4": (s4h, 4), "5": (s4h, 5)}
        seq = []
        for ch in order:
            if ch == "0":
                seq.append(("s0", 0))
            elif ch == "1":
                seq.append(("s1", 1))
            elif ch == "2":
                seq.append(("s2", 2))
            elif ch == "3":
                seq.append(("s3", 3))
            elif ch == "4":
                seq.append(("s4a", 4))
            elif ch == "5":
                seq.append(("s4b", 5))
        fns = {"s0": s0, "s1": s1, "s2": s2, "s3": s3,
               "s4a": lambda i: s4h(i, 0), "s4b": lambda i: s4h(i, 1)}
        for k in range(T + 5):
            for name, lag in seq:
                i = k - lag
                if 0 <= i < T:
                    fns[name](i)

    return nc


def _get_program(general_lnw=False, general_pb=False):
    key = ("nc", general_lnw, general_pb)
    if key not in _prog_cache:
        nc = _build_program(general_lnw, general_pb)
        _split_excess_waits(nc)
        _prog_cache[key] = nc
    return _prog_cache[key]


def kernel(**inputs):
    x = np.ascontiguousarray(inputs["x"], dtype=np.float32)
    x_list = np.ascontiguousarray(inputs["x_list"], dtype=np.float32)
    f_w = np.asarray(inputs["f_w"], dtype=np.float32)
    f_b = np.asarray(inputs["f_b"], dtype=np.float32)
    h_w = np.asarray(inputs["h_w"], dtype=np.float32)
    h_b = np.asarray(inputs["h_b"], dtype=np.float32)
    proj_w = np.asarray(inputs["proj_w"], dtype=np.float32)
    proj_b = np.asarray(inputs["proj_b"], dtype=np.float32)
    ln_w = np.asarray(inputs["ln_w"], dtype=np.float32)
    ln_b = np.asarray(inputs["ln_b"], dtype=np.float32)

    general_lnw = not np.allclose(ln_w, 1.0)
    general_pb = not np.allclose(proj_b, 0.0)

    # ---- host-side weight prep (tiny) ----
    # fx stationary [97, 195] per j; col order: q outs 0..95, gates, q outs
    # 96..191 (so the M-split 0:99 / 99:195 keeps slices contiguous).
    fwj = []
    for j in range(2):
        a = np.zeros((97, 195), dtype=np.float32)
        blk = f_w[:, 96 * j:96 * (j + 1)]           # [195 outs, 96 ins_j]
        a[0:96, 0:96] = blk[0:96].T
        a[0:96, 96:99] = blk[192:195].T
        a[0:96, 99:195] = blk[96:192].T
        if j == 0:
            a[96, 0:96] = f_b[0:96]
            a[96, 96:99] = f_b[192:195]
            a[96, 99:195] = f_b[96:192]
        fwj.append(a.astype(np.float16))
    hwj = [np.ascontiguousarray(h_w[:, 96 * j:96 * (j + 1)].T).astype(
        np.float16) for j in range(2)]
    w_mu = proj_w.mean(axis=0)
    pw = proj_w - w_mu[None, :]                     # mean-folded [out, in]
    pjj = [np.ascontiguousarray(pw[:, 96 * j:96 * (j + 1)].T).astype(
        np.float16) for j in range(2)]
    pbt = (proj_b - proj_b.mean())[None, :].astype(np.float16)
    hbv = np.ascontiguousarray(h_b.reshape(2, 96).T).astype(np.float32)

    # ---- host-side input prep ----
    xs = x.reshape(NCORES, NS, C, HW)
    xa = np.empty((NCORES, NS, 194, HW), dtype=np.float16)
    xa[:, :, 0:96] = xs[:, :, 0:96]
    xa[:, :, 96] = 1.0
    xa[:, :, 97:193] = xs[:, :, 96:192]
    xa[:, :, 193] = 1.0
    xls = np.ascontiguousarray(
        x_list.reshape(L, NCORES, NS, C, HW).transpose(0, 1, 2, 4, 3)
    ).astype(np.float16)                            # [L, NC, NS, HW, C]
    x0p = xs.transpose(0, 1, 3, 2) + ln_b[None, None, None, :]
    x0p = np.ascontiguousarray(x0p).astype(np.float16)  # [NC, NS, HW, C]

    common = {
        "fw0": fwj[0], "fw1": fwj[1],
        "hw0": hwj[0], "hw1": hwj[1],
        "pj0": pjj[0], "pj1": pjj[1],
        "pbt": pbt, "hb": hbv,
        "id16": np.eye(128, dtype=np.float16),
        "id32": np.concatenate([np.eye(3, dtype=np.float16),
                                np.zeros((96, 3), np.float16)], axis=0),
        "one1": np.ones((1, 128), dtype=np.float16),
        "epsc": np.full((128, 1), EPS, dtype=np.float32),
    }
    if general_lnw:
        common["lwp"] = np.ascontiguousarray(
            np.broadcast_to(ln_w[None, :], (128, 192))).astype(np.float16)
    in_maps = []
    for c in range(NCORES):
        m = dict(common)
        m["x"] = xa[c]
        m["x0p"] = x0p[c]
        for l in range(L):
            m[f"xl{l}"] = xls[l, c]
        in_maps.append(m)

    nc = _get_program(general_lnw, general_pb)
    res = run_bass_kernel_spmd(nc, in_maps, core_ids=list(range(NCORES)))
    out = np.stack([r["out"] for r in res.results], axis=0)  # [NC,NS,HW,C]
    out = out.astype(np.float32).transpose(0, 1, 3, 2)       # [NC,NS,C,HW]
    return np.ascontiguousarray(out.reshape(N_FULL, C, H, W))


# revision 7
# speedup vs baseline: 1.0357x; 1.0357x over previous
"""Trainium2 Bass kernel for the focal-modulation dense_cnn problem (v5).

Math (per reference):
  fx = conv1x1(x, f_w, f_b);  q, gates = fx[:, :C], fx[:, C:]
  ctx = sum_l x_list[l] * gates[:, l]
  mod = conv1x1(ctx, h_w, h_b)
  y   = conv1x1(q * mod, proj_w, proj_b)
  out = layernorm_c(y) * ln_w + ln_b + x

Strategy (data-parallel, 2 batches/core, 8 cores; F=512-pixel tiles):
  * fx runs channel-major ([97,2,F] moving, fp16) with the 3 gate channels
    folded into the stationary (M=99 piece) -- 4 matmuls.
  * Gates are transposed on the PE ([3,128] -> [128,3] per 128-px group) so
    the gating multiply becomes DVE tensor_scalar with per-partition fp32
    scalars (4x perf mode on fp16 SBUF operands): 12 ts + 2 adds per tile.
  * ctx is transposed back to channel-major on the PE (8 tiny transposes)
    and copied PSUM->SBUF; mod is 4 channel-major matmuls; xo=(mod+hb)*q is
    scalar_tensor_tensor on Pool.
  * proj uses xo as the *stationary* operand ([96,128] chunks) with the
    weight matrix moving, so y lands pixel-major [128pix,192ch] in PSUM
    (8 matmuls of 192 rows + 4 bias-broadcast matmuls) -- the LayerNorm
    tail then needs no partition reductions: var = Act Square+accum_out,
    istd is a per-partition scalar, apply = stt((y*istd) + x0') with the
    residual and ln bias folded into a host-precomputed pixel-major x0'.
  * All activation tensors fp16 (DMA bytes halved); matmuls fp16 (1 cyc/row).
"""

import sys

sys.path.insert(0, "/opt/trn_rl_repo")

import numpy as np

import bass_rust
import concourse.bass as bass
import concourse.mybir as mybir
import concourse.tile as tile
from concourse.bass_utils import run_bass_kernel_spmd
from concourse.vector_clock import ScopedClock

# ---------------------------------------------------------------------------
# Workaround: this walrus build accepts only one sem wait per instruction
# ("Too many sync wait commands"). (1) chain the Tile tail drain's waits;
# (2) post-pass that moves excess waits onto NoOps inserted just before the
# offending instruction on the same engine.


def _patched_drain_and_barrier(self, tick_clock, wait_clock):
    nc = self.nc
    drain_inst = nc.sync.drain()
    wait_clock.add_sem_waits(
        drain_inst.ins, ScopedClock({None: tick_clock.global_clock})
    )
    si = drain_inst.ins.sync_info
    if si is not None and len(si.on_wait) > 1:
        waits = list(si.on_wait)
        drain_inst.ins.sync_info = bass_rust.SyncInfo(
            on_wait=waits[:1], on_update=list(si.on_update)
        )
        for w in waits[1:]:
            d2 = nc.sync.drain()
            d2.ins.sync_info = bass_rust.SyncInfo(on_wait=[w], on_update=[])
    nc.all_engine_barrier()
    assert self.sems is not None
    popped = nc._tile_sem_poison_stack.pop()
    assert popped is self._sem_poison
    nc.clear_and_free_semaphores(list(self.sems.allocated().values()))
    nc.all_engine_barrier()


tile.TileContext._drain_and_barrier = _patched_drain_and_barrier

_WAIT_LIMIT = 1


def _split_excess_waits(nc):
    k = 0
    for f in nc.m.functions:
        for b in f.blocks:
            il = b.instructions
            new = []
            for inst in il:
                si = inst.sync_info
                if si is not None and len(si.on_wait) > _WAIT_LIMIT:
                    waits = list(si.on_wait)
                    excess, keep = waits[:-_WAIT_LIMIT], waits[-_WAIT_LIMIT:]
                    for w in excess:
                        nop = mybir.InstNoOp(name=f"wsplit-{k}",
                                             engine=inst.engine)
                        nop.sync_info = bass_rust.SyncInfo(on_wait=[w],
                                                           on_update=[])
                        new.append(nop)
                        k += 1
                    inst.sync_info = bass_rust.SyncInfo(
                        on_wait=keep, on_update=list(si.on_update))
                new.append(inst)
            il[:] = new
    return k
# ---------------------------------------------------------------------------

FP32 = mybir.dt.float32
F16 = mybir.dt.float16
AF = mybir.ActivationFunctionType
OP = mybir.AluOpType

NCORES = 8
N_FULL, C, H, W, L = 16, 192, 128, 128, 3
HW = H * W
NS = N_FULL // NCORES          # batch per core
MAC = 1024                     # pixels per DMA macro-tile
F = 512                        # pixels per inner tile
NMAC = HW // MAC
NF = MAC // F
NCH = F // 256                 # 256-px chunks per F tile (=2)
EPS = 1e-6
RSC = float(1.0 / np.sqrt(C))  # variance via Square(in*RSC) accumulation

_prog_cache = {}


def _ileave(base):
    """[P, 256] contiguous AP -> [P, (par 2)(idx 128)] interleaved view.

    Free enumeration (par, idx) maps position par*128+idx to element
    par + 2*idx, so channel-major pixel order within each 256-px chunk
    matches the pixel-major packing (partition p <-> pixel 2p+par).
    """
    return bass_rust.AP(tensor=base.tensor, offset=base.offset,
                        ap=[base.ap[0], [1, 2], [2, 128]])


def _build_program(general_lnw, general_pb):
    nc = bass.Bass(trn_type="TRN2")

    d_x = nc.dram_tensor("x", [NS, 194, HW], F16, kind="ExternalInput")
    d_xl = [nc.dram_tensor(f"xl{l}", [NS, HW, C], F16, kind="ExternalInput")
            for l in range(L)]
    d_x0 = nc.dram_tensor("x0p", [NS, HW, C], F16, kind="ExternalInput")
    d_fw = [nc.dram_tensor(f"fw{j}", [97, 195], F16, kind="ExternalInput")
            for j in range(2)]
    d_hw = [nc.dram_tensor(f"hw{j}", [96, 192], F16, kind="ExternalInput")
            for j in range(2)]
    d_pj = [nc.dram_tensor(f"pj{j}", [96, 192], F16, kind="ExternalInput")
            for j in range(2)]
    d_pbt = nc.dram_tensor("pbt", [1, 192], F16, kind="ExternalInput")
    d_hb = nc.dram_tensor("hb", [96, 2], FP32, kind="ExternalInput")
    d_id16 = nc.dram_tensor("id16", [128, 128], F16, kind="ExternalInput")
    d_id32 = nc.dram_tensor("id32", [99, 3], F16, kind="ExternalInput")
    d_one1 = nc.dram_tensor("one1", [1, 128], F16, kind="ExternalInput")
    d_eps = nc.dram_tensor("epsc", [128, 1], FP32, kind="ExternalInput")
    if general_lnw:
        d_lwp = nc.dram_tensor("lwp", [128, 192], F16, kind="ExternalInput")
    d_out = nc.dram_tensor("out", [NS, HW, C], F16, kind="ExternalOutput")

    # x: [NS, 194, HW] -> [NS][97, 2, HW]; aug channel a = 97*j + p
    vx = d_x[:, :, :].rearrange("n (j p) w -> n p j w", j=2)
    # pixel-major tensors: pixel = 1024*m + 256*cc + 2*p + par
    vxl = [t[:, :, :].rearrange("n (m cc p par) c -> n m p cc (par c)",
                                m=NMAC, cc=4, p=128, par=2) for t in d_xl]
    vx0 = d_x0[:, :, :].rearrange("n (m cc p par) c -> n m p cc (par c)",
                                  m=NMAC, cc=4, p=128, par=2)
    vout = d_out[:, :, :].rearrange("n (m cc p par) c -> n m p cc (par c)",
                                    m=NMAC, cc=4, p=128, par=2)

    from contextlib import ExitStack
    with tile.TileContext(nc) as tc, ExitStack() as ctx:
        sing = ctx.enter_context(tc.tile_pool(name="sing", bufs=1))
        inp = ctx.enter_context(tc.tile_pool(name="inp", bufs=4))
        qgp = ctx.enter_context(tc.tile_pool(name="qgp", bufs=4))
        gat = ctx.enter_context(tc.tile_pool(name="gat", bufs=2))
        csb = ctx.enter_context(tc.tile_pool(name="csb", bufs=2))
        xop = ctx.enter_context(tc.tile_pool(name="xop", bufs=3))
        tlp = ctx.enter_context(tc.tile_pool(name="tlp", bufs=2))
        zp = ctx.enter_context(tc.tile_pool(name="zp", bufs=3))
        pfx = ctx.enter_context(tc.tile_pool(name="pfx", bufs=1, space="PSUM"))
        pgp = ctx.enter_context(tc.tile_pool(name="pgp", bufs=1, space="PSUM"))
        pcx = ctx.enter_context(tc.tile_pool(name="pcx", bufs=1, space="PSUM"))
        pmd = ctx.enter_context(tc.tile_pool(name="pmd", bufs=1, space="PSUM"))
        ppj = ctx.enter_context(tc.tile_pool(name="ppj", bufs=2, space="PSUM"))

        # ---- load constants / weights (once) ----
        def loadw(dram, shape, dt, tag):
            t = sing.tile(shape, dt, tag=tag, name=tag)
            nc.sync.dma_start(out=t,
                              in_=dram[tuple(slice(0, s) for s in shape)])
            return t

        fw = [loadw(d_fw[j], [97, 195], F16, f"fw{j}") for j in range(2)]
        hwt = [loadw(d_hw[j], [96, 192], F16, f"hw{j}") for j in range(2)]
        pjc = [loadw(d_pj[j], [96, 192], F16, f"pj{j}") for j in range(2)]
        pbt = loadw(d_pbt, [1, 192], F16, "pbt")
        hb = loadw(d_hb, [96, 2], FP32, "hb")
        id16 = loadw(d_id16, [128, 128], F16, "id16")
        id32 = loadw(d_id32, [99, 3], F16, "id32")
        one1 = loadw(d_one1, [1, 128], F16, "one1")
        eps_t = loadw(d_eps, [128, 1], FP32, "epsc")
        lwp = loadw(d_lwp, [128, 192], F16, "lwp") if general_lnw else None

        mm = nc.tensor.matmul
        tiles = [(n, im, fi) for n in range(NS) for im in range(NMAC)
                 for fi in range(NF)]
        T = len(tiles)
        macs = {}
        st = {}

        def load_macro(n, im):
            o0 = im * MAC
            x_t = inp.tile([97, 2, MAC], F16, tag="x", name="x_t")
            nc.sync.dma_start(out=x_t, in_=vx[n, :, :, o0:o0 + MAC])
            xl_t = []
            for l in range(L):
                t = inp.tile([128, 4, 2 * C], F16, tag=f"xl{l}",
                             name=f"xl{l}")
                nc.sync.dma_start(out=t, in_=vxl[l][n, im])
                xl_t.append(t)
            x0_t = inp.tile([128, 4, 2 * C], F16, tag="x0", name="x0_t")
            nc.sync.dma_start(out=x0_t, in_=vx0[n, im])
            macs[(n, im)] = (x_t, xl_t, x0_t)

        # ---- stage S0: DMA + fx matmuls + q/gates PSUM->SBUF copies ----
        def _xmov(x_t, j, fi):
            """fx moving operand: [97, F] with free order (cc, par, idx)
            matching the pixel-major group packing."""
            base = x_t[:, j, fi * F:(fi + 1) * F]
            return bass_rust.AP(tensor=base.tensor, offset=base.offset,
                                ap=[base.ap[0], [256, 2], [1, 2], [2, 128]])

        def s0(i):
            n, im, fi = tiles[i]
            if (n, im) not in macs:
                load_macro(n, im)
            x_t, xl_t, x0_t = macs[(n, im)]
            fx = pfx.tile([99, 2, F], FP32, tag="fx", name="fx")
            for j in range(2):
                mm(fx[:, 0, :], fw[j][:, 0:99], _xmov(x_t, j, fi),
                   start=(j == 0), stop=(j == 1))
            for j in range(2):
                mm(fx[0:96, 1, :], fw[j][:, 99:195], _xmov(x_t, j, fi),
                   start=(j == 0), stop=(j == 1))
            # q + gates to SBUF in one copy (fp32: gate transposes/ts
            # scalars need fp32)
            qg = qgp.tile([99, 2, F], F16, tag="qg", name="qg")
            nc.scalar.activation(qg, fx, AF.Copy)
            st[i] = {"n": n, "im": im, "fi": fi,
                     "xl_t": xl_t, "x0_t": x0_t, "qg": qg}

        # ---- stage S1: gate transposes + gating (pixel-major) ----
        def s1(i):
            d = st[i]
            fi = d["fi"]
            qg0 = d["qg"][:, 0, :]
            xl_t = d["xl_t"]
            gts = gat.tile([3, F], F16, tag="gts", name="gts")
            nc.sync.dma_start(out=gts, in_=qg0[96:99, :])
            gps = pgp.tile([128, 4, 4], F16, tag="gps", name="gps")
            for cp in range(4):
                nc.tensor.transpose(gps[:, cp, 0:3],
                                    gts[:, cp * 128:(cp + 1) * 128],
                                    id32[0:3, :])
            gpsf = gat.tile([128, 4, 4], FP32, tag="gpsf", name="gpsf")
            nc.vector.tensor_scalar(gpsf, gps, 1.0, None, OP.mult)
            t0 = gat.tile([128, 4, C], F16, tag="t0", name="t0")
            t1 = gat.tile([128, 4, C], F16, tag="t1", name="t1")
            t2 = gat.tile([128, 4, C], F16, tag="t2", name="t2")
            tl = [t0, t1, t2]
            for l in range(L):
                for cc in range(NCH):
                    for par in range(2):
                        cp = 2 * cc + par
                        eng = nc.vector if l < 2 else nc.gpsimd
                        eng.tensor_scalar(
                            tl[l][:, cp, :],
                            xl_t[l][:, 2 * fi + cc,
                                    par * C:(par + 1) * C],
                            gpsf[:, cp, l:l + 1], None, OP.mult)
            cxa = gat.tile([128, 4, C], F16, tag="cxa", name="cxa")
            nc.vector.tensor_tensor(cxa, t0, t1, OP.add)
            cxb = gat.tile([128, 4, C], F16, tag="cxb", name="cxb")
            nc.vector.tensor_tensor(cxb, cxa, t2, OP.add)
            d["ctx_px"] = cxb

        # ---- stage S2: ctx transposes back to channel-major + copy ----
        def s2(i):
            d = st[i]
            cxb = d["ctx_px"]
            ccm = pcx.tile([96, 2, F], F16, tag="ccm", name="ccm")
            for j in range(2):
                for cp in range(4):
                    nc.tensor.transpose(
                        ccm[:, j, cp * 128:(cp + 1) * 128],
                        cxb[:, cp, 96 * j:96 * (j + 1)], id16)
            csb_t = csb.tile([96, 2, F], F16, tag="csb", name="csb_t")
            nc.vector.tensor_scalar(csb_t, ccm, 1.0, None, OP.mult)
            d["ctx_sb"] = csb_t

        # ---- stage S3: mod matmuls + xo = (mod+hb)*q on Pool ----
        def s3(i):
            d = st[i]
            ctx_sb = d["ctx_sb"]
            mod = pmd.tile([96, 2, F], FP32, tag="mod", name="mod")
            for m in range(2):
                for j in range(2):
                    mm(mod[:, m, :], hwt[j][:, 96 * m:96 * (m + 1)],
                       ctx_sb[:, j, :], start=(j == 0), stop=(j == 1))
            msb = xop.tile([96, 2, F], F16, tag="msb", name="msb")
            for m in range(2):
                nc.scalar.activation(msb[:, m, :], mod[:, m, :],
                                     AF.Identity, bias=hb[:, m:m + 1])
            xo = xop.tile([96, 2, F], F16, tag="xo", name="xo")
            nc.vector.tensor_tensor(xo, msb, d["qg"][0:96, :, :], OP.mult)
            d["xo"] = xo

        # ---- stage S4a/S4b: proj (pixel-major out) + LN tail, one half
        # (= one 256-px chunk pair... 2 of the 4 groups) per step so the
        # pj PSUM pool double-buffers within the bank budget ----
        zmacs = {}

        def s4h(i, h):
            d = st[i]
            xo = d["xo"]
            x0_t = d["x0_t"]
            n, im, fi = d["n"], d["im"], d["fi"]
            if (n, im) not in zmacs:
                zmacs[(n, im)] = zp.tile([128, 4, 2 * C], F16, tag="zmac",
                                         name="zmac")
            zmac = zmacs[(n, im)]
            pjh = ppj.tile([128, 2, 192], FP32, tag="pjh", name="pjh")
            for ci in range(2):
                cp = 2 * h + ci
                out = pjh[:, ci, :]
                for j in range(2):
                    mm(out, xo[:, j, cp * 128:(cp + 1) * 128], pjc[j],
                       start=(j == 0), stop=(j == 1 and not general_pb))
                if general_pb:
                    mm(out, one1, pbt, start=False, stop=True)
            var2 = tlp.tile([128, 2], FP32, tag="var2", name="var2")
            sqs = tlp.tile([128, 2, 192], F16, tag="sqs", name="sqs")
            nc.scalar.activation(sqs, pjh, AF.Square, scale=RSC)
            for ci in range(2):
                nc.vector.tensor_reduce(var2[:, ci:ci + 1], sqs[:, ci, :],
                                        mybir.AxisListType.X, OP.add)
            lnv = tlp.tile([128, 2], FP32, tag="lnv", name="lnv")
            nc.scalar.activation(lnv, var2, AF.Ln, bias=eps_t)
            istd = tlp.tile([128, 2], FP32, tag="istd", name="istd")
            nc.scalar.activation(istd, lnv, AF.Exp, scale=-0.5)
            z1 = tlp.tile([128, 2 * C], F16, tag="z1", name="z1")
            for ci in range(2):
                nc.vector.tensor_scalar(z1[:, ci * C:(ci + 1) * C],
                                        pjh[:, ci, :],
                                        istd[:, ci:ci + 1], None, OP.mult)
                if general_lnw:
                    nc.vector.tensor_tensor(z1[:, ci * C:(ci + 1) * C],
                                            z1[:, ci * C:(ci + 1) * C],
                                            lwp, OP.mult)
            nc.vector.tensor_tensor(zmac[:, 2 * fi + h, :], z1,
                                    x0_t[:, 2 * fi + h, :], OP.add)
            if h == 1:
                st.pop(i)
                if fi == NF - 1:
                    nc.sync.dma_start(out=vout[n, im], in_=zmac)
                    del zmacs[(n, im)]
                    del macs[(n, im)]

        # ---- software pipeline: 6 slots ----
        import os
        order = os.environ.get("K5_ORDER", "345102")
        slots = {"0": (s0, 0), "1": (s1, 1), "2": (s2, 2), "3": (s3, 3),
                 "4": (s4h, 4), "5": (s4h, 5)}
        seq = []
        for ch in order:
            if ch == "0":
                seq.append(("s0", 0))
            elif ch == "1":
                seq.append(("s1", 1))
            elif ch == "2":
                seq.append(("s2", 2))
            elif ch == "3":
                seq.append(("s3", 3))
            elif ch == "4":
                seq.append(("s4a", 4))
            elif ch == "5":
                seq.append(("s4b", 5))
        fns = {"s0": s0, "s1": s1, "s2": s2, "s3": s3,
               "s4a": lambda i: s4h(i, 0), "s4b": lambda i: s4h(i, 1)}
        for k in range(T + 5):
            for name, lag in seq:
                i = k - lag
                if 0 <= i < T:
                    fns[name](i)

    return nc


def _get_program(general_lnw=False, general_pb=False):
    key = ("nc", general_lnw, general_pb)
    if key not in _prog_cache:
        nc = _build_program(general_lnw, general_pb)
        _split_excess_waits(nc)
        _prog_cache[key] = nc
    return _prog_cache[key]


def kernel(**inputs):
    x = np.ascontiguousarray(inputs["x"], dtype=np.float32)
    x_list = np.ascontiguousarray(inputs["x_list"], dtype=np.float32)
    f_w = np.asarray(inputs["f_w"], dtype=np.float32)
    f_b = np.asarray(inputs["f_b"], dtype=np.float32)
    h_w = np.asarray(inputs["h_w"], dtype=np.float32)
    h_b = np.asarray(inputs["h_b"], dtype=np.float32)
    proj_w = np.asarray(inputs["proj_w"], dtype=np.float32)
    proj_b = np.asarray(inputs["proj_b"], dtype=np.float32)
    ln_w = np.asarray(inputs["ln_w"], dtype=np.float32)
    ln_b = np.asarray(inputs["ln_b"], dtype=np.float32)

    general_lnw = not np.allclose(ln_w, 1.0)
    general_pb = not np.allclose(proj_b, 0.0)

    # ---- host-side weight prep (tiny) ----
    # fx stationary [97, 195] per j; col order: q outs 0..95, gates, q outs
    # 96..191 (so the M-split 0:99 / 99:195 keeps slices contiguous).
    fwj = []
    for j in range(2):
        a = np.zeros((97, 195), dtype=np.float32)
        blk = f_w[:, 96 * j:96 * (j + 1)]           # [195 outs, 96 ins_j]
        a[0:96, 0:96] = blk[0:96].T
        a[0:96, 96:99] = blk[192:195].T
        a[0:96, 99:195] = blk[96:192].T
        if j == 0:
            a[96, 0:96] = f_b[0:96]
            a[96, 96:99] = f_b[192:195]
            a[96, 99:195] = f_b[96:192]
        fwj.append(a.astype(np.float16))
    hwj = [np.ascontiguousarray(h_w[:, 96 * j:96 * (j + 1)].T).astype(
        np.float16) for j in range(2)]
    w_mu = proj_w.mean(axis=0)
    pw = proj_w - w_mu[None, :]                     # mean-folded [out, in]
    pjj = [np.ascontiguousarray(pw[:, 96 * j:96 * (j + 1)].T).astype(
        np.float16) for j in range(2)]
    pbt = (proj_b - proj_b.mean())[None, :].astype(np.float16)
    hbv = np.ascontiguousarray(h_b.reshape(2, 96).T).astype(np.float32)

    # ---- host-side input prep ----
    xs = x.reshape(NCORES, NS, C, HW)
    xa = np.empty((NCORES, NS, 194, HW), dtype=np.float16)
    xa[:, :, 0:96] = xs[:, :, 0:96]
    xa[:, :, 96] = 1.0
    xa[:, :, 97:193] = xs[:, :, 96:192]
    xa[:, :, 193] = 1.0
    xls = np.ascontiguousarray(
        x_list.reshape(L, NCORES, NS, C, HW).transpose(0, 1, 2, 4, 3)
    ).astype(np.float16)                            # [L, NC, NS, HW, C]
    x0p = xs.transpose(0, 1, 3, 2) + ln_b[None, None, None, :]
    x0p = np.ascontiguousarray(x0p).astype(np.float16)  # [NC, NS, HW, C]

    common = {
        "fw0": fwj[0], "fw1": fwj[1],
        "hw0": hwj[0], "hw1": hwj[1],
        "pj0": pjj[0], "pj1": pjj[1],
        "pbt": pbt, "hb": hbv,
        "id16": np.eye(128, dtype=np.float16),
        "id32": np.concatenate([np.eye(3, dtype=np.float16),
                                np.zeros((96, 3), np.float16)], axis=0),
        "one1": np.ones((1, 128), dtype=np.float16),
        "epsc": np.full((128, 1), EPS, dtype=np.float32),
    }
    if general_lnw:
        common["lwp"] = np.ascontiguousarray(
            np.broadcast_to(ln_w[None, :], (128, 192))).astype(np.float16)
    in_maps = []
    for c in range(NCORES):
        m = dict(common)
        m["x"] = xa[c]
        m["x0p"] = x0p[c]
        for l in range(L):
            m[f"xl{l}"] = xls[l, c]
        in_maps.append(m)

    nc = _get_program(general_lnw, general_pb)
    res = run_bass_kernel_spmd(nc, in_maps, core_ids=list(range(NCORES)))
    out = np.stack([r["out"] for r in res.results], axis=0)  # [NC,NS,HW,C]
    out = out.astype(np.float32).transpose(0, 1, 3, 2)       # [NC,NS,C,HW]
    return np.ascontiguousarray(out.reshape(N_FULL, C, H, W))


# revision 13
# speedup vs baseline: 1.0831x; 1.0457x over previous
"""Trainium2 Bass kernel for the focal-modulation dense_cnn problem (v5).

Math (per reference):
  fx = conv1x1(x, f_w, f_b);  q, gates = fx[:, :C], fx[:, C:]
  ctx = sum_l x_list[l] * gates[:, l]
  mod = conv1x1(ctx, h_w, h_b)
  y   = conv1x1(q * mod, proj_w, proj_b)
  out = layernorm_c(y) * ln_w + ln_b + x

Strategy (data-parallel, 2 batches/core, 8 cores; F=512-pixel tiles):
  * fx runs channel-major ([97,2,F] moving, fp16) with the 3 gate channels
    folded into the stationary (M=99 piece) -- 4 matmuls.
  * Gates are transposed on the PE ([3,128] -> [128,3] per 128-px group) so
    the gating multiply becomes DVE tensor_scalar with per-partition fp32
    scalars (4x perf mode on fp16 SBUF operands): 12 ts + 2 adds per tile.
  * ctx is transposed back to channel-major on the PE (8 tiny transposes)
    and copied PSUM->SBUF; mod is 4 channel-major matmuls; xo=(mod+hb)*q is
    scalar_tensor_tensor on Pool.
  * proj uses xo as the *stationary* operand ([96,128] chunks) with the
    weight matrix moving, so y lands pixel-major [128pix,192ch] in PSUM
    (8 matmuls of 192 rows + 4 bias-broadcast matmuls) -- the LayerNorm
    tail then needs no partition reductions: var = Act Square+accum_out,
    istd is a per-partition scalar, apply = stt((y*istd) + x0') with the
    residual and ln bias folded into a host-precomputed pixel-major x0'.
  * All activation tensors fp16 (DMA bytes halved); matmuls fp16 (1 cyc/row).
"""

import sys

sys.path.insert(0, "/opt/trn_rl_repo")

import numpy as np

import bass_rust
import concourse.bass as bass
import concourse.mybir as mybir
import concourse.tile as tile
from concourse.bass_utils import run_bass_kernel_spmd
from concourse.vector_clock import ScopedClock

# ---------------------------------------------------------------------------
# Workaround: this walrus build accepts only one sem wait per instruction
# ("Too many sync wait commands"). (1) chain the Tile tail drain's waits;
# (2) post-pass that moves excess waits onto NoOps inserted just before the
# offending instruction on the same engine.


def _patched_drain_and_barrier(self, tick_clock, wait_clock):
    nc = self.nc
    drain_inst = nc.sync.drain()
    wait_clock.add_sem_waits(
        drain_inst.ins, ScopedClock({None: tick_clock.global_clock})
    )
    si = drain_inst.ins.sync_info
    if si is not None and len(si.on_wait) > 1:
        waits = list(si.on_wait)
        drain_inst.ins.sync_info = bass_rust.SyncInfo(
            on_wait=waits[:1], on_update=list(si.on_update)
        )
        for w in waits[1:]:
            d2 = nc.sync.drain()
            d2.ins.sync_info = bass_rust.SyncInfo(on_wait=[w], on_update=[])
    nc.all_engine_barrier()
    assert self.sems is not None
    popped = nc._tile_sem_poison_stack.pop()
    assert popped is self._sem_poison
    nc.clear_and_free_semaphores(list(self.sems.allocated().values()))
    nc.all_engine_barrier()


tile.TileContext._drain_and_barrier = _patched_drain_and_barrier

_WAIT_LIMIT = 1


def _split_excess_waits(nc):
    k = 0
    for f in nc.m.functions:
        for b in f.blocks:
            il = b.instructions
            new = []
            for inst in il:
                si = inst.sync_info
                if si is not None and len(si.on_wait) > _WAIT_LIMIT:
                    waits = list(si.on_wait)
                    excess, keep = waits[:-_WAIT_LIMIT], waits[-_WAIT_LIMIT:]
                    for w in excess:
                        nop = mybir.InstNoOp(name=f"wsplit-{k}",
                                             engine=inst.engine)
                        nop.sync_info = bass_rust.SyncInfo(on_wait=[w],
                                                           on_update=[])
                        new.append(nop)
                        k += 1
                    inst.sync_info = bass_rust.SyncInfo(
                        on_wait=keep, on_update=list(si.on_update))
                new.append(inst)
            il[:] = new
    return k
# ---------------------------------------------------------------------------

FP32 = mybir.dt.float32
F16 = mybir.dt.float16
AF = mybir.ActivationFunctionType
OP = mybir.AluOpType

NCORES = 8
N_FULL, C, H, W, L = 16, 192, 128, 128, 3
HW = H * W
NS = N_FULL // NCORES          # batch per core
MAC = 1024                     # pixels per DMA macro-tile
F = 512                        # pixels per inner tile
NMAC = HW // MAC
NF = MAC // F
NCH = F // 256                 # 256-px chunks per F tile (=2)
EPS = 1e-6
RSC = float(1.0 / np.sqrt(C))  # variance via Square(in*RSC) accumulation

_prog_cache = {}


def _ileave(base):
    """[P, 256] contiguous AP -> [P, (par 2)(idx 128)] interleaved view.

    Free enumeration (par, idx) maps position par*128+idx to element
    par + 2*idx, so channel-major pixel order within each 256-px chunk
    matches the pixel-major packing (partition p <-> pixel 2p+par).
    """
    return bass_rust.AP(tensor=base.tensor, offset=base.offset,
                        ap=[base.ap[0], [1, 2], [2, 128]])


def _build_program(general_lnw, general_pb):
    nc = bass.Bass(trn_type="TRN2")

    d_x = nc.dram_tensor("x", [NS, 194, HW], F16, kind="ExternalInput")
    d_xl = [nc.dram_tensor(f"xl{l}", [NS, HW, C], F16, kind="ExternalInput")
            for l in range(L)]
    d_x0 = nc.dram_tensor("x0p", [NS, HW, C], F16, kind="ExternalInput")
    d_fw = [nc.dram_tensor(f"fw{j}", [97, 195], F16, kind="ExternalInput")
            for j in range(2)]
    d_hw = [nc.dram_tensor(f"hw{j}", [96, 192], F16, kind="ExternalInput")
            for j in range(2)]
    d_pj = [nc.dram_tensor(f"pj{j}", [96, 192], F16, kind="ExternalInput")
            for j in range(2)]
    d_pbt = nc.dram_tensor("pbt", [1, 192], F16, kind="ExternalInput")
    d_hb = nc.dram_tensor("hb", [96, 2], FP32, kind="ExternalInput")
    d_id16 = nc.dram_tensor("id16", [128, 128], F16, kind="ExternalInput")
    d_id32 = nc.dram_tensor("id32", [99, 3], F16, kind="ExternalInput")
    d_one1 = nc.dram_tensor("one1", [1, 128], F16, kind="ExternalInput")
    d_eps = nc.dram_tensor("epsc", [128, 1], FP32, kind="ExternalInput")
    if general_lnw:
        d_lwp = nc.dram_tensor("lwp", [128, 192], F16, kind="ExternalInput")
    d_out = nc.dram_tensor("out", [NS, HW, C], F16, kind="ExternalOutput")

    # x: [NS, 194, HW] -> [NS][97, 2, HW]; aug channel a = 97*j + p
    vx = d_x[:, :, :].rearrange("n (j p) w -> n p j w", j=2)
    # pixel-major tensors: pixel = 1024*m + 256*cc + 2*p + par
    vxl = [t[:, :, :].rearrange("n (m cc p par) c -> n m p cc (par c)",
                                m=NMAC, cc=4, p=128, par=2) for t in d_xl]
    vx0 = d_x0[:, :, :].rearrange("n (m cc p par) c -> n m p cc (par c)",
                                  m=NMAC, cc=4, p=128, par=2)
    vout = d_out[:, :, :].rearrange("n (m cc p par) c -> n m p cc (par c)",
                                    m=NMAC, cc=4, p=128, par=2)

    from contextlib import ExitStack
    with tile.TileContext(nc) as tc, ExitStack() as ctx:
        sing = ctx.enter_context(tc.tile_pool(name="sing", bufs=1))
        inp = ctx.enter_context(tc.tile_pool(name="inp", bufs=4))
        qgp = ctx.enter_context(tc.tile_pool(name="qgp", bufs=4))
        gat = ctx.enter_context(tc.tile_pool(name="gat", bufs=2))
        csb = ctx.enter_context(tc.tile_pool(name="csb", bufs=2))
        xop = ctx.enter_context(tc.tile_pool(name="xop", bufs=3))
        tlp = ctx.enter_context(tc.tile_pool(name="tlp", bufs=2))
        zp = ctx.enter_context(tc.tile_pool(name="zp", bufs=3))
        pfx = ctx.enter_context(tc.tile_pool(name="pfx", bufs=1, space="PSUM"))
        pgp = ctx.enter_context(tc.tile_pool(name="pgp", bufs=1, space="PSUM"))
        pcx = ctx.enter_context(tc.tile_pool(name="pcx", bufs=1, space="PSUM"))
        pmd = ctx.enter_context(tc.tile_pool(name="pmd", bufs=1, space="PSUM"))
        ppj = ctx.enter_context(tc.tile_pool(name="ppj", bufs=2, space="PSUM"))

        # ---- load constants / weights (once) ----
        def loadw(dram, shape, dt, tag):
            t = sing.tile(shape, dt, tag=tag, name=tag)
            nc.sync.dma_start(out=t,
                              in_=dram[tuple(slice(0, s) for s in shape)])
            return t

        fw = [loadw(d_fw[j], [97, 195], F16, f"fw{j}") for j in range(2)]
        hwt = [loadw(d_hw[j], [96, 192], F16, f"hw{j}") for j in range(2)]
        pjc = [loadw(d_pj[j], [96, 192], F16, f"pj{j}") for j in range(2)]
        pbt = loadw(d_pbt, [1, 192], F16, "pbt")
        hb = loadw(d_hb, [96, 2], FP32, "hb")
        id16 = loadw(d_id16, [128, 128], F16, "id16")
        id32 = loadw(d_id32, [99, 3], F16, "id32")
        one1 = loadw(d_one1, [1, 128], F16, "one1")
        eps_t = loadw(d_eps, [128, 1], FP32, "epsc")
        lwp = loadw(d_lwp, [128, 192], F16, "lwp") if general_lnw else None

        mm = nc.tensor.matmul
        tiles = [(n, im, fi) for n in range(NS) for im in range(NMAC)
                 for fi in range(NF)]
        T = len(tiles)
        macs = {}
        st = {}

        def load_macro(n, im):
            o0 = im * MAC
            x_t = inp.tile([97, 2, MAC], F16, tag="x", name="x_t")
            nc.sync.dma_start(out=x_t, in_=vx[n, :, :, o0:o0 + MAC])
            xl_t = []
            for l in range(L):
                t = inp.tile([128, 4, 2 * C], F16, tag=f"xl{l}",
                             name=f"xl{l}")
                nc.sync.dma_start(out=t, in_=vxl[l][n, im])
                xl_t.append(t)
            x0_t = inp.tile([128, 4, 2 * C], F16, tag="x0", name="x0_t")
            nc.sync.dma_start(out=x0_t, in_=vx0[n, im])
            macs[(n, im)] = (x_t, xl_t, x0_t)

        # ---- stage S0: DMA + fx matmuls + q/gates PSUM->SBUF copies ----
        def _xmov(x_t, j, fi):
            """fx moving operand: [97, F] with free order (cc, par, idx)
            matching the pixel-major group packing."""
            base = x_t[:, j, fi * F:(fi + 1) * F]
            return bass_rust.AP(tensor=base.tensor, offset=base.offset,
                                ap=[base.ap[0], [256, 2], [1, 2], [2, 128]])

        def s0(i):
            n, im, fi = tiles[i]
            if (n, im) not in macs:
                load_macro(n, im)
            x_t, xl_t, x0_t = macs[(n, im)]
            fx = pfx.tile([99, 2, F], FP32, tag="fx", name="fx")
            for j in range(2):
                mm(fx[:, 0, :], fw[j][:, 0:99], _xmov(x_t, j, fi),
                   start=(j == 0), stop=(j == 1))
            for j in range(2):
                mm(fx[0:96, 1, :], fw[j][:, 99:195], _xmov(x_t, j, fi),
                   start=(j == 0), stop=(j == 1))
            # q + gates to SBUF in one copy (fp32: gate transposes/ts
            # scalars need fp32)
            qg = qgp.tile([99, 2, F], F16, tag="qg", name="qg")
            nc.scalar.activation(qg, fx, AF.Copy)
            st[i] = {"n": n, "im": im, "fi": fi,
                     "xl_t": xl_t, "x0_t": x0_t, "qg": qg}

        # ---- stage S1: gate transposes + gating (pixel-major) ----
        def s1(i):
            d = st[i]
            fi = d["fi"]
            qg0 = d["qg"][:, 0, :]
            xl_t = d["xl_t"]
            gts = gat.tile([3, F], F16, tag="gts", name="gts")
            nc.sync.dma_start(out=gts, in_=qg0[96:99, :])
            gps = pgp.tile([128, 4, 4], F16, tag="gps", name="gps")
            for cp in range(4):
                nc.tensor.transpose(gps[:, cp, 0:3],
                                    gts[:, cp * 128:(cp + 1) * 128],
                                    id32[0:3, :])
            gpsf = gat.tile([128, 4, 4], FP32, tag="gpsf", name="gpsf")
            nc.vector.tensor_scalar(gpsf, gps, 1.0, None, OP.mult)
            t0 = gat.tile([128, 4, C], F16, tag="t0", name="t0")
            t1 = gat.tile([128, 4, C], F16, tag="t1", name="t1")
            t2 = gat.tile([128, 4, C], F16, tag="t2", name="t2")
            tl = [t0, t1, t2]
            for l in range(L):
                for cc in range(NCH):
                    for par in range(2):
                        cp = 2 * cc + par
                        eng = nc.vector if l < 2 else nc.gpsimd
                        eng.tensor_scalar(
                            tl[l][:, cp, :],
                            xl_t[l][:, 2 * fi + cc,
                                    par * C:(par + 1) * C],
                            gpsf[:, cp, l:l + 1], None, OP.mult)
            cxa = gat.tile([128, 4, C], F16, tag="cxa", name="cxa")
            nc.vector.tensor_tensor(cxa, t0, t1, OP.add)
            cxb = gat.tile([128, 4, C], F16, tag="cxb", name="cxb")
            nc.vector.tensor_tensor(cxb, cxa, t2, OP.add)
            d["ctx_px"] = cxb

        # ---- stage S2: ctx transposes back to channel-major + copy ----
        def s2(i):
            d = st[i]
            cxb = d["ctx_px"]
            ccm = pcx.tile([96, 2, F], F16, tag="ccm", name="ccm")
            for j in range(2):
                for cp in range(4):
                    nc.tensor.transpose(
                        ccm[:, j, cp * 128:(cp + 1) * 128],
                        cxb[:, cp, 96 * j:96 * (j + 1)], id16)
            csb_t = csb.tile([96, 2, F], F16, tag="csb", name="csb_t")
            nc.vector.tensor_scalar(csb_t, ccm, 1.0, None, OP.mult)
            d["ctx_sb"] = csb_t

        # ---- stage S3: mod matmuls + xo = (mod+hb)*q on Pool ----
        def s3(i):
            d = st[i]
            ctx_sb = d["ctx_sb"]
            mod = pmd.tile([96, 2, F], FP32, tag="mod", name="mod")
            for m in range(2):
                for j in range(2):
                    mm(mod[:, m, :], hwt[j][:, 96 * m:96 * (m + 1)],
                       ctx_sb[:, j, :], start=(j == 0), stop=(j == 1))
            msb = xop.tile([96, 2, F], F16, tag="msb", name="msb")
            for m in range(2):
                nc.scalar.activation(msb[:, m, :], mod[:, m, :],
                                     AF.Identity, bias=hb[:, m:m + 1])
            xo = xop.tile([96, 2, F], F16, tag="xo", name="xo")
            nc.vector.tensor_tensor(xo, msb, d["qg"][0:96, :, :], OP.mult)
            d["xo"] = xo

        # ---- stage S4a/S4b: proj (pixel-major out) + LN tail, one half
        # (= one 256-px chunk pair... 2 of the 4 groups) per step so the
        # pj PSUM pool double-buffers within the bank budget ----
        zmacs = {}

        def s4h(i, h):
            d = st[i]
            xo = d["xo"]
            x0_t = d["x0_t"]
            n, im, fi = d["n"], d["im"], d["fi"]
            if (n, im) not in zmacs:
                zmacs[(n, im)] = zp.tile([128, 4, 2 * C], F16, tag="zmac",
                                         name="zmac")
            zmac = zmacs[(n, im)]
            pjh = ppj.tile([128, 2, 192], FP32, tag="pjh", name="pjh")
            for ci in range(2):
                cp = 2 * h + ci
                out = pjh[:, ci, :]
                for j in range(2):
                    mm(out, xo[:, j, cp * 128:(cp + 1) * 128], pjc[j],
                       start=(j == 0), stop=(j == 1 and not general_pb))
                if general_pb:
                    mm(out, one1, pbt, start=False, stop=True)
            var2 = tlp.tile([128, 2], FP32, tag="var2", name="var2")
            sqs = tlp.tile([128, 2, 192], F16, tag="sqs", name="sqs")
            nc.scalar.activation(sqs, pjh, AF.Square, scale=RSC)
            for ci in range(2):
                nc.vector.tensor_reduce(var2[:, ci:ci + 1], sqs[:, ci, :],
                                        mybir.AxisListType.X, OP.add)
            lnv = tlp.tile([128, 2], FP32, tag="lnv", name="lnv")
            nc.scalar.activation(lnv, var2, AF.Ln, bias=eps_t)
            istd = tlp.tile([128, 2], FP32, tag="istd", name="istd")
            nc.scalar.activation(istd, lnv, AF.Exp, scale=-0.5)
            z1 = tlp.tile([128, 2 * C], F16, tag="z1", name="z1")
            nc.scalar.activation(z1[:, 0:C], pjh[:, 0, :], AF.Identity,
                                 scale=istd[:, 0:1])
            nc.vector.tensor_scalar(z1[:, C:2 * C], pjh[:, 1, :],
                                    istd[:, 1:2], None, OP.mult)
            if general_lnw:
                for ci in range(2):
                    nc.vector.tensor_tensor(z1[:, ci * C:(ci + 1) * C],
                                            z1[:, ci * C:(ci + 1) * C],
                                            lwp, OP.mult)
            nc.vector.tensor_tensor(zmac[:, 2 * fi + h, :], z1,
                                    x0_t[:, 2 * fi + h, :], OP.add)
            if h == 1:
                st.pop(i)
                if fi == NF - 1:
                    nc.sync.dma_start(out=vout[n, im], in_=zmac)
                    del zmacs[(n, im)]
                    del macs[(n, im)]

        # ---- software pipeline: 6 slots ----
        import os
        order = os.environ.get("K5_ORDER", "345102")
        slots = {"0": (s0, 0), "1": (s1, 1), "2": (s2, 2), "3": (s3, 3),
                 "4": (s4h, 4), "5": (s4h, 5)}
        seq = []
        for ch in order:
            if ch == "0":
                seq.append(("s0", 0))
            elif ch == "1":
                seq.append(("s1", 1))
            elif ch == "2":
                seq.append(("s2", 2))
            elif ch == "3":
                seq.append(("s3", 3))
            elif ch == "4":
                seq.append(("s4a", 4))
            elif ch == "5":
                seq.append(("s4b", 5))
        fns = {"s0": s0, "s1": s1, "s2": s2, "s3": s3,
               "s4a": lambda i: s4h(i, 0), "s4b": lambda i: s4h(i, 1)}
        for k in range(T + 5):
            for name, lag in seq:
                i = k - lag
                if 0 <= i < T:
                    fns[name](i)

    return nc


def _get_program(general_lnw=False, general_pb=False):
    key = ("nc", general_lnw, general_pb)
    if key not in _prog_cache:
        nc = _build_program(general_lnw, general_pb)
        _split_excess_waits(nc)
        _prog_cache[key] = nc
    return _prog_cache[key]


def kernel(**inputs):
    x = np.ascontiguousarray(inputs["x"], dtype=np.float32)
    x_list = np.ascontiguousarray(inputs["x_list"], dtype=np.float32)
    f_w = np.asarray(inputs["f_w"], dtype=np.float32)
    f_b = np.asarray(inputs["f_b"], dtype=np.float32)
    h_w = np.asarray(inputs["h_w"], dtype=np.float32)
    h_b = np.asarray(inputs["h_b"], dtype=np.float32)
    proj_w = np.asarray(inputs["proj_w"], dtype=np.float32)
    proj_b = np.asarray(inputs["proj_b"], dtype=np.float32)
    ln_w = np.asarray(inputs["ln_w"], dtype=np.float32)
    ln_b = np.asarray(inputs["ln_b"], dtype=np.float32)

    general_lnw = not np.allclose(ln_w, 1.0)
    general_pb = not np.allclose(proj_b, 0.0)

    # ---- host-side weight prep (tiny) ----
    # fx stationary [97, 195] per j; col order: q outs 0..95, gates, q outs
    # 96..191 (so the M-split 0:99 / 99:195 keeps slices contiguous).
    fwj = []
    for j in range(2):
        a = np.zeros((97, 195), dtype=np.float32)
        blk = f_w[:, 96 * j:96 * (j + 1)]           # [195 outs, 96 ins_j]
        a[0:96, 0:96] = blk[0:96].T
        a[0:96, 96:99] = blk[192:195].T
        a[0:96, 99:195] = blk[96:192].T
        if j == 0:
            a[96, 0:96] = f_b[0:96]
            a[96, 96:99] = f_b[192:195]
            a[96, 99:195] = f_b[96:192]
        fwj.append(a.astype(np.float16))
    hwj = [np.ascontiguousarray(h_w[:, 96 * j:96 * (j + 1)].T).astype(
        np.float16) for j in range(2)]
    w_mu = proj_w.mean(axis=0)
    pw = proj_w - w_mu[None, :]                     # mean-folded [out, in]
    pjj = [np.ascontiguousarray(pw[:, 96 * j:96 * (j + 1)].T).astype(
        np.float16) for j in range(2)]
    pbt = (proj_b - proj_b.mean())[None, :].astype(np.float16)
    hbv = np.ascontiguousarray(h_b.reshape(2, 96).T).astype(np.float32)

    # ---- host-side input prep ----
    xs = x.reshape(NCORES, NS, C, HW)
    xa = np.empty((NCORES, NS, 194, HW), dtype=np.float16)
    xa[:, :, 0:96] = xs[:, :, 0:96]
    xa[:, :, 96] = 1.0
    xa[:, :, 97:193] = xs[:, :, 96:192]
    xa[:, :, 193] = 1.0
    xls = np.ascontiguousarray(
        x_list.reshape(L, NCORES, NS, C, HW).transpose(0, 1, 2, 4, 3)
    ).astype(np.float16)                            # [L, NC, NS, HW, C]
    x0p = xs.transpose(0, 1, 3, 2) + ln_b[None, None, None, :]
    x0p = np.ascontiguousarray(x0p).astype(np.float16)  # [NC, NS, HW, C]

    common = {
        "fw0": fwj[0], "fw1": fwj[1],
        "hw0": hwj[0], "hw1": hwj[1],
        "pj0": pjj[0], "pj1": pjj[1],
        "pbt": pbt, "hb": hbv,
        "id16": np.eye(128, dtype=np.float16),
        "id32": np.concatenate([np.eye(3, dtype=np.float16),
                                np.zeros((96, 3), np.float16)], axis=0),
        "one1": np.ones((1, 128), dtype=np.float16),
        "epsc": np.full((128, 1), EPS, dtype=np.float32),
    }
    if general_lnw:
        common["lwp"] = np.ascontiguousarray(
            np.broadcast_to(ln_w[None, :], (128, 192))).astype(np.float16)
    in_maps = []
    for c in range(NCORES):
        m = dict(common)
        m["x"] = xa[c]
        m["x0p"] = x0p[c]
        for l in range(L):
            m[f"xl{l}"] = xls[l, c]
        in_maps.append(m)

    nc = _get_program(general_lnw, general_pb)
    res = run_bass_kernel_spmd(nc, in_maps, core_ids=list(range(NCORES)))
    out = np.stack([r["out"] for r in res.results], axis=0)  # [NC,NS,HW,C]
    out = out.astype(np.float32).transpose(0, 1, 3, 2)       # [NC,NS,C,HW]
    return np.ascontiguousarray(out.reshape(N_FULL, C, H, W))


# revision 15
# speedup vs baseline: 1.1058x; 1.0210x over previous
"""Trainium2 Bass kernel for the focal-modulation dense_cnn problem (v5).

Math (per reference):
  fx = conv1x1(x, f_w, f_b);  q, gates = fx[:, :C], fx[:, C:]
  ctx = sum_l x_list[l] * gates[:, l]
  mod = conv1x1(ctx, h_w, h_b)
  y   = conv1x1(q * mod, proj_w, proj_b)
  out = layernorm_c(y) * ln_w + ln_b + x

Strategy (data-parallel, 2 batches/core, 8 cores; F=512-pixel tiles):
  * fx runs channel-major ([97,2,F] moving, fp16) with the 3 gate channels
    folded into the stationary (M=99 piece) -- 4 matmuls.
  * Gates are transposed on the PE ([3,128] -> [128,3] per 128-px group) so
    the gating multiply becomes DVE tensor_scalar with per-partition fp32
    scalars (4x perf mode on fp16 SBUF operands): 12 ts + 2 adds per tile.
  * ctx is transposed back to channel-major on the PE (8 tiny transposes)
    and copied PSUM->SBUF; mod is 4 channel-major matmuls; xo=(mod+hb)*q is
    scalar_tensor_tensor on Pool.
  * proj uses xo as the *stationary* operand ([96,128] chunks) with the
    weight matrix moving, so y lands pixel-major [128pix,192ch] in PSUM
    (8 matmuls of 192 rows + 4 bias-broadcast matmuls) -- the LayerNorm
    tail then needs no partition reductions: var = Act Square+accum_out,
    istd is a per-partition scalar, apply = stt((y*istd) + x0') with the
    residual and ln bias folded into a host-precomputed pixel-major x0'.
  * All activation tensors fp16 (DMA bytes halved); matmuls fp16 (1 cyc/row).
"""

import sys

sys.path.insert(0, "/opt/trn_rl_repo")

import numpy as np

import bass_rust
import concourse.bass as bass
import concourse.mybir as mybir
import concourse.tile as tile
from concourse.bass_utils import run_bass_kernel_spmd
from concourse.vector_clock import ScopedClock

# ---------------------------------------------------------------------------
# Workaround: this walrus build accepts only one sem wait per instruction
# ("Too many sync wait commands"). (1) chain the Tile tail drain's waits;
# (2) post-pass that moves excess waits onto NoOps inserted just before the
# offending instruction on the same engine.


def _patched_drain_and_barrier(self, tick_clock, wait_clock):
    nc = self.nc
    drain_inst = nc.sync.drain()
    wait_clock.add_sem_waits(
        drain_inst.ins, ScopedClock({None: tick_clock.global_clock})
    )
    si = drain_inst.ins.sync_info
    if si is not None and len(si.on_wait) > 1:
        waits = list(si.on_wait)
        drain_inst.ins.sync_info = bass_rust.SyncInfo(
            on_wait=waits[:1], on_update=list(si.on_update)
        )
        for w in waits[1:]:
            d2 = nc.sync.drain()
            d2.ins.sync_info = bass_rust.SyncInfo(on_wait=[w], on_update=[])
    nc.all_engine_barrier()
    assert self.sems is not None
    popped = nc._tile_sem_poison_stack.pop()
    assert popped is self._sem_poison
    nc.clear_and_free_semaphores(list(self.sems.allocated().values()))
    nc.all_engine_barrier()


tile.TileContext._drain_and_barrier = _patched_drain_and_barrier

_WAIT_LIMIT = 1


def _split_excess_waits(nc):
    k = 0
    for f in nc.m.functions:
        for b in f.blocks:
            il = b.instructions
            new = []
            for inst in il:
                si = inst.sync_info
                if si is not None and len(si.on_wait) > _WAIT_LIMIT:
                    waits = list(si.on_wait)
                    excess, keep = waits[:-_WAIT_LIMIT], waits[-_WAIT_LIMIT:]
                    for w in excess:
                        nop = mybir.InstNoOp(name=f"wsplit-{k}",
                                             engine=inst.engine)
                        nop.sync_info = bass_rust.SyncInfo(on_wait=[w],
                                                           on_update=[])
                        new.append(nop)
                        k += 1
                    inst.sync_info = bass_rust.SyncInfo(
                        on_wait=keep, on_update=list(si.on_update))
                new.append(inst)
            il[:] = new
    return k
# ---------------------------------------------------------------------------

FP32 = mybir.dt.float32
F16 = mybir.dt.float16
AF = mybir.ActivationFunctionType
OP = mybir.AluOpType

NCORES = 8
N_FULL, C, H, W, L = 16, 192, 128, 128, 3
HW = H * W
NS = N_FULL // NCORES          # batch per core
MAC = 1024                     # pixels per DMA macro-tile
F = 512                        # pixels per inner tile
NMAC = HW // MAC
NF = MAC // F
NCH = F // 256                 # 256-px chunks per F tile (=2)
EPS = 1e-6
RSC = float(1.0 / np.sqrt(C))  # variance via Square(in*RSC) accumulation

_prog_cache = {}


def _ileave(base):
    """[P, 256] contiguous AP -> [P, (par 2)(idx 128)] interleaved view.

    Free enumeration (par, idx) maps position par*128+idx to element
    par + 2*idx, so channel-major pixel order within each 256-px chunk
    matches the pixel-major packing (partition p <-> pixel 2p+par).
    """
    return bass_rust.AP(tensor=base.tensor, offset=base.offset,
                        ap=[base.ap[0], [1, 2], [2, 128]])


def _build_program(general_lnw, general_pb):
    nc = bass.Bass(trn_type="TRN2")

    d_x = nc.dram_tensor("x", [NS, 194, HW], F16, kind="ExternalInput")
    d_xl = [nc.dram_tensor(f"xl{l}", [NS, HW, C], F16, kind="ExternalInput")
            for l in range(L)]
    d_x0 = nc.dram_tensor("x0p", [NS, HW, C], F16, kind="ExternalInput")
    d_fw = [nc.dram_tensor(f"fw{j}", [97, 195], F16, kind="ExternalInput")
            for j in range(2)]
    d_hw = [nc.dram_tensor(f"hw{j}", [96, 192], F16, kind="ExternalInput")
            for j in range(2)]
    d_pj = [nc.dram_tensor(f"pj{j}", [96, 192], F16, kind="ExternalInput")
            for j in range(2)]
    d_pbt = nc.dram_tensor("pbt", [1, 192], F16, kind="ExternalInput")
    d_hb = nc.dram_tensor("hb", [96, 2], FP32, kind="ExternalInput")
    d_id16 = nc.dram_tensor("id16", [128, 128], F16, kind="ExternalInput")
    d_id32 = nc.dram_tensor("id32", [99, 3], F16, kind="ExternalInput")
    d_one1 = nc.dram_tensor("one1", [1, 128], F16, kind="ExternalInput")
    d_eps = nc.dram_tensor("epsc", [128, 1], FP32, kind="ExternalInput")
    if general_lnw:
        d_lwp = nc.dram_tensor("lwp", [128, 192], F16, kind="ExternalInput")
    d_out = nc.dram_tensor("out", [NS, HW, C], F16, kind="ExternalOutput")

    # x: [NS, 194, HW] -> [NS][97, 2, HW]; aug channel a = 97*j + p
    vx = d_x[:, :, :].rearrange("n (j p) w -> n p j w", j=2)
    # pixel-major tensors: pixel = 1024*m + 256*cc + 2*p + par
    vxl = [t[:, :, :].rearrange("n (m cc p par) c -> n m p cc (par c)",
                                m=NMAC, cc=4, p=128, par=2) for t in d_xl]
    vx0 = d_x0[:, :, :].rearrange("n (m cc p par) c -> n m p cc (par c)",
                                  m=NMAC, cc=4, p=128, par=2)
    vout = d_out[:, :, :].rearrange("n (m cc p par) c -> n m p cc (par c)",
                                    m=NMAC, cc=4, p=128, par=2)

    from contextlib import ExitStack
    with tile.TileContext(nc) as tc, ExitStack() as ctx:
        sing = ctx.enter_context(tc.tile_pool(name="sing", bufs=1))
        inp = ctx.enter_context(tc.tile_pool(name="inp", bufs=4))
        qgp = ctx.enter_context(tc.tile_pool(name="qgp", bufs=4))
        gat = ctx.enter_context(tc.tile_pool(name="gat", bufs=2))
        csb = ctx.enter_context(tc.tile_pool(name="csb", bufs=2))
        xop = ctx.enter_context(tc.tile_pool(name="xop", bufs=3))
        tlp = ctx.enter_context(tc.tile_pool(name="tlp", bufs=2))
        zp = ctx.enter_context(tc.tile_pool(name="zp", bufs=3))
        pfx = ctx.enter_context(tc.tile_pool(name="pfx", bufs=1, space="PSUM"))
        pgp = ctx.enter_context(tc.tile_pool(name="pgp", bufs=1, space="PSUM"))
        pcx = ctx.enter_context(tc.tile_pool(name="pcx", bufs=1, space="PSUM"))
        pmd = ctx.enter_context(tc.tile_pool(name="pmd", bufs=1, space="PSUM"))
        ppj = ctx.enter_context(tc.tile_pool(name="ppj", bufs=2, space="PSUM"))

        # ---- load constants / weights (once) ----
        def loadw(dram, shape, dt, tag):
            t = sing.tile(shape, dt, tag=tag, name=tag)
            nc.sync.dma_start(out=t,
                              in_=dram[tuple(slice(0, s) for s in shape)])
            return t

        fw = [loadw(d_fw[j], [97, 195], F16, f"fw{j}") for j in range(2)]
        hwt = [loadw(d_hw[j], [96, 192], F16, f"hw{j}") for j in range(2)]
        pjc = [loadw(d_pj[j], [96, 192], F16, f"pj{j}") for j in range(2)]
        pbt = loadw(d_pbt, [1, 192], F16, "pbt")
        hb = loadw(d_hb, [96, 2], FP32, "hb")
        id16 = loadw(d_id16, [128, 128], F16, "id16")
        id32 = loadw(d_id32, [99, 3], F16, "id32")
        one1 = loadw(d_one1, [1, 128], F16, "one1")
        eps_t = loadw(d_eps, [128, 1], FP32, "epsc")
        lwp = loadw(d_lwp, [128, 192], F16, "lwp") if general_lnw else None

        mm = nc.tensor.matmul
        tiles = [(n, im, fi) for n in range(NS) for im in range(NMAC)
                 for fi in range(NF)]
        T = len(tiles)
        macs = {}
        st = {}

        def load_macro(n, im):
            o0 = im * MAC
            x_t = inp.tile([97, 2, MAC], F16, tag="x", name="x_t")
            nc.sync.dma_start(out=x_t, in_=vx[n, :, :, o0:o0 + MAC])
            xl_t = []
            for l in range(L):
                t = inp.tile([128, 4, 2 * C], F16, tag=f"xl{l}",
                             name=f"xl{l}")
                nc.sync.dma_start(out=t, in_=vxl[l][n, im])
                xl_t.append(t)
            x0_t = inp.tile([128, 4, 2 * C], F16, tag="x0", name="x0_t")
            nc.sync.dma_start(out=x0_t, in_=vx0[n, im])
            macs[(n, im)] = (x_t, xl_t, x0_t)

        # ---- stage S0: DMA + fx matmuls + q/gates PSUM->SBUF copies ----
        def _xmov(x_t, j, fi):
            """fx moving operand: [97, F] with free order (cc, par, idx)
            matching the pixel-major group packing."""
            base = x_t[:, j, fi * F:(fi + 1) * F]
            return bass_rust.AP(tensor=base.tensor, offset=base.offset,
                                ap=[base.ap[0], [256, 2], [1, 2], [2, 128]])

        def s0(i):
            n, im, fi = tiles[i]
            if (n, im) not in macs:
                load_macro(n, im)
            x_t, xl_t, x0_t = macs[(n, im)]
            fx = pfx.tile([99, 2, F], FP32, tag="fx", name="fx")
            for j in range(2):
                mm(fx[:, 0, :], fw[j][:, 0:99], _xmov(x_t, j, fi),
                   start=(j == 0), stop=(j == 1))
            for j in range(2):
                mm(fx[0:96, 1, :], fw[j][:, 99:195], _xmov(x_t, j, fi),
                   start=(j == 0), stop=(j == 1))
            # q + gates to SBUF in one copy (fp32: gate transposes/ts
            # scalars need fp32)
            qg = qgp.tile([99, 2, F], F16, tag="qg", name="qg")
            nc.scalar.activation(qg, fx, AF.Copy)
            st[i] = {"n": n, "im": im, "fi": fi,
                     "xl_t": xl_t, "x0_t": x0_t, "qg": qg}

        # ---- stage S1: gate transposes + gating (pixel-major) ----
        def s1(i):
            d = st[i]
            fi = d["fi"]
            qg0 = d["qg"][:, 0, :]
            xl_t = d["xl_t"]
            gts = gat.tile([3, F], F16, tag="gts", name="gts")
            nc.sync.dma_start(out=gts, in_=qg0[96:99, :])
            gps = pgp.tile([128, 4, 4], F16, tag="gps", name="gps")
            for cp in range(4):
                nc.tensor.transpose(gps[:, cp, 0:3],
                                    gts[:, cp * 128:(cp + 1) * 128],
                                    id32[0:3, :])
            gpsf = gat.tile([128, 4, 4], FP32, tag="gpsf", name="gpsf")
            nc.vector.tensor_scalar(gpsf, gps, 1.0, None, OP.mult)
            t0 = gat.tile([128, 4, C], F16, tag="t0", name="t0")
            t1 = gat.tile([128, 4, C], F16, tag="t1", name="t1")
            t2 = gat.tile([128, 4, C], F16, tag="t2", name="t2")
            tl = [t0, t1, t2]
            for l in range(L):
                for cc in range(NCH):
                    for par in range(2):
                        cp = 2 * cc + par
                        eng = nc.vector if l < 2 else nc.gpsimd
                        eng.tensor_scalar(
                            tl[l][:, cp, :],
                            xl_t[l][:, 2 * fi + cc,
                                    par * C:(par + 1) * C],
                            gpsf[:, cp, l:l + 1], None, OP.mult)
            cxa = gat.tile([128, 4, C], F16, tag="cxa", name="cxa")
            nc.vector.tensor_tensor(cxa, t0, t1, OP.add)
            cxb = gat.tile([128, 4, C], F16, tag="cxb", name="cxb")
            nc.vector.tensor_tensor(cxb, cxa, t2, OP.add)
            d["ctx_px"] = cxb

        # ---- stage S2: ctx transposes back to channel-major + copy ----
        def s2(i):
            d = st[i]
            cxb = d["ctx_px"]
            ccm = pcx.tile([96, 2, F], F16, tag="ccm", name="ccm")
            for j in range(2):
                for cp in range(4):
                    nc.tensor.transpose(
                        ccm[:, j, cp * 128:(cp + 1) * 128],
                        cxb[:, cp, 96 * j:96 * (j + 1)], id16)
            csb_t = csb.tile([96, 2, F], F16, tag="csb", name="csb_t")
            nc.vector.tensor_scalar(csb_t, ccm, 1.0, None, OP.mult)
            d["ctx_sb"] = csb_t

        # ---- stage S3: mod matmuls + xo = (mod+hb)*q on Pool ----
        def s3(i):
            d = st[i]
            ctx_sb = d["ctx_sb"]
            mod = pmd.tile([96, 2, F], FP32, tag="mod", name="mod")
            for m in range(2):
                for j in range(2):
                    mm(mod[:, m, :], hwt[j][:, 96 * m:96 * (m + 1)],
                       ctx_sb[:, j, :], start=(j == 0), stop=(j == 1))
            msb = xop.tile([96, 2, F], F16, tag="msb", name="msb")
            for m in range(2):
                nc.scalar.activation(msb[:, m, :], mod[:, m, :],
                                     AF.Identity, bias=hb[:, m:m + 1])
            xo = xop.tile([96, 2, F], F16, tag="xo", name="xo")
            nc.vector.tensor_tensor(xo, msb, d["qg"][0:96, :, :], OP.mult)
            d["xo"] = xo

        # ---- stage S4a/S4b: proj (pixel-major out) + LN tail, one half
        # (= one 256-px chunk pair... 2 of the 4 groups) per step so the
        # pj PSUM pool double-buffers within the bank budget ----
        zmacs = {}

        def s4h(i, h):
            d = st[i]
            xo = d["xo"]
            x0_t = d["x0_t"]
            n, im, fi = d["n"], d["im"], d["fi"]
            if (n, im) not in zmacs:
                zmacs[(n, im)] = zp.tile([128, 4, 2 * C], F16, tag="zmac",
                                         name="zmac")
            zmac = zmacs[(n, im)]
            pjh = ppj.tile([128, 2, 192], FP32, tag="pjh", name="pjh")
            for ci in range(2):
                cp = 2 * h + ci
                out = pjh[:, ci, :]
                for j in range(2):
                    mm(out, xo[:, j, cp * 128:(cp + 1) * 128], pjc[j],
                       start=(j == 0), stop=(j == 1 and not general_pb))
                if general_pb:
                    mm(out, one1, pbt, start=False, stop=True)
            var2 = tlp.tile([128, 2], FP32, tag="var2", name="var2")
            sqs = tlp.tile([128, 2, 192], F16, tag="sqs", name="sqs")
            nc.scalar.activation(sqs, pjh, AF.Square, scale=RSC)
            nc.vector.tensor_reduce(var2, sqs,
                                    mybir.AxisListType.X, OP.add)
            lnv = tlp.tile([128, 2], FP32, tag="lnv", name="lnv")
            nc.scalar.activation(lnv, var2, AF.Ln, bias=eps_t)
            istd = tlp.tile([128, 2], FP32, tag="istd", name="istd")
            nc.scalar.activation(istd, lnv, AF.Exp, scale=-0.5)
            z1 = tlp.tile([128, 2 * C], F16, tag="z1", name="z1")
            nc.scalar.activation(z1[:, 0:C], pjh[:, 0, :], AF.Identity,
                                 scale=istd[:, 0:1])
            nc.vector.tensor_scalar(z1[:, C:2 * C], pjh[:, 1, :],
                                    istd[:, 1:2], None, OP.mult)
            if general_lnw:
                for ci in range(2):
                    nc.vector.tensor_tensor(z1[:, ci * C:(ci + 1) * C],
                                            z1[:, ci * C:(ci + 1) * C],
                                            lwp, OP.mult)
            nc.vector.tensor_tensor(zmac[:, 2 * fi + h, :], z1,
                                    x0_t[:, 2 * fi + h, :], OP.add)
            if h == 1:
                st.pop(i)
                if fi == NF - 1:
                    nc.sync.dma_start(out=vout[n, im], in_=zmac)
                    del zmacs[(n, im)]
                    del macs[(n, im)]

        # ---- software pipeline: 6 slots ----
        import os
        order = os.environ.get("K5_ORDER", "345102")
        slots = {"0": (s0, 0), "1": (s1, 1), "2": (s2, 2), "3": (s3, 3),
                 "4": (s4h, 4), "5": (s4h, 5)}
        seq = []
        for ch in order:
            if ch == "0":
                seq.append(("s0", 0))
            elif ch == "1":
                seq.append(("s1", 1))
            elif ch == "2":
                seq.append(("s2", 2))
            elif ch == "3":
                seq.append(("s3", 3))
            elif ch == "4":
                seq.append(("s4a", 4))
            elif ch == "5":
                seq.append(("s4b", 5))
        fns = {"s0": s0, "s1": s1, "s2": s2, "s3": s3,
               "s4a": lambda i: s4h(i, 0), "s4b": lambda i: s4h(i, 1)}
        for k in range(T + 5):
            for name, lag in seq:
                i = k - lag
                if 0 <= i < T:
                    fns[name](i)

    return nc


def _get_program(general_lnw=False, general_pb=False):
    key = ("nc", general_lnw, general_pb)
    if key not in _prog_cache:
        nc = _build_program(general_lnw, general_pb)
        _split_excess_waits(nc)
        _prog_cache[key] = nc
    return _prog_cache[key]


def kernel(**inputs):
    x = np.ascontiguousarray(inputs["x"], dtype=np.float32)
    x_list = np.ascontiguousarray(inputs["x_list"], dtype=np.float32)
    f_w = np.asarray(inputs["f_w"], dtype=np.float32)
    f_b = np.asarray(inputs["f_b"], dtype=np.float32)
    h_w = np.asarray(inputs["h_w"], dtype=np.float32)
    h_b = np.asarray(inputs["h_b"], dtype=np.float32)
    proj_w = np.asarray(inputs["proj_w"], dtype=np.float32)
    proj_b = np.asarray(inputs["proj_b"], dtype=np.float32)
    ln_w = np.asarray(inputs["ln_w"], dtype=np.float32)
    ln_b = np.asarray(inputs["ln_b"], dtype=np.float32)

    general_lnw = not np.allclose(ln_w, 1.0)
    general_pb = not np.allclose(proj_b, 0.0)

    # ---- host-side weight prep (tiny) ----
    # fx stationary [97, 195] per j; col order: q outs 0..95, gates, q outs
    # 96..191 (so the M-split 0:99 / 99:195 keeps slices contiguous).
    fwj = []
    for j in range(2):
        a = np.zeros((97, 195), dtype=np.float32)
        blk = f_w[:, 96 * j:96 * (j + 1)]           # [195 outs, 96 ins_j]
        a[0:96, 0:96] = blk[0:96].T
        a[0:96, 96:99] = blk[192:195].T
        a[0:96, 99:195] = blk[96:192].T
        if j == 0:
            a[96, 0:96] = f_b[0:96]
            a[96, 96:99] = f_b[192:195]
            a[96, 99:195] = f_b[96:192]
        fwj.append(a.astype(np.float16))
    hwj = [np.ascontiguousarray(h_w[:, 96 * j:96 * (j + 1)].T).astype(
        np.float16) for j in range(2)]
    w_mu = proj_w.mean(axis=0)
    pw = proj_w - w_mu[None, :]                     # mean-folded [out, in]
    pjj = [np.ascontiguousarray(pw[:, 96 * j:96 * (j + 1)].T).astype(
        np.float16) for j in range(2)]
    pbt = (proj_b - proj_b.mean())[None, :].astype(np.float16)
    hbv = np.ascontiguousarray(h_b.reshape(2, 96).T).astype(np.float32)

    # ---- host-side input prep ----
    xs = x.reshape(NCORES, NS, C, HW)
    xa = np.empty((NCORES, NS, 194, HW), dtype=np.float16)
    xa[:, :, 0:96] = xs[:, :, 0:96]
    xa[:, :, 96] = 1.0
    xa[:, :, 97:193] = xs[:, :, 96:192]
    xa[:, :, 193] = 1.0
    xls = np.ascontiguousarray(
        x_list.reshape(L, NCORES, NS, C, HW).transpose(0, 1, 2, 4, 3)
    ).astype(np.float16)                            # [L, NC, NS, HW, C]
    x0p = xs.transpose(0, 1, 3, 2) + ln_b[None, None, None, :]
    x0p = np.ascontiguousarray(x0p).astype(np.float16)  # [NC, NS, HW, C]

    common = {
        "fw0": fwj[0], "fw1": fwj[1],
        "hw0": hwj[0], "hw1": hwj[1],
        "pj0": pjj[0], "pj1": pjj[1],
        "pbt": pbt, "hb": hbv,
        "id16": np.eye(128, dtype=np.float16),
        "id32": np.concatenate([np.eye(3, dtype=np.float16),
                                np.zeros((96, 3), np.float16)], axis=0),
        "one1": np.ones((1, 128), dtype=np.float16),
        "epsc": np.full((128, 1), EPS, dtype=np.float32),
    }
    if general_lnw:
        common["lwp"] = np.ascontiguousarray(
            np.broadcast_to(ln_w[None, :], (128, 192))).astype(np.float16)
    in_maps = []
    for c in range(NCORES):
        m = dict(common)
        m["x"] = xa[c]
        m["x0p"] = x0p[c]
        for l in range(L):
            m[f"xl{l}"] = xls[l, c]
        in_maps.append(m)

    nc = _get_program(general_lnw, general_pb)
    res = run_bass_kernel_spmd(nc, in_maps, core_ids=list(range(NCORES)))
    out = np.stack([r["out"] for r in res.results], axis=0)  # [NC,NS,HW,C]
    out = out.astype(np.float32).transpose(0, 1, 3, 2)       # [NC,NS,C,HW]
    return np.ascontiguousarray(out.reshape(N_FULL, C, H, W))


# revision 16
# speedup vs baseline: 1.1322x; 1.0239x over previous
"""Trainium2 Bass kernel for the focal-modulation dense_cnn problem (v5).

Math (per reference):
  fx = conv1x1(x, f_w, f_b);  q, gates = fx[:, :C], fx[:, C:]
  ctx = sum_l x_list[l] * gates[:, l]
  mod = conv1x1(ctx, h_w, h_b)
  y   = conv1x1(q * mod, proj_w, proj_b)
  out = layernorm_c(y) * ln_w + ln_b + x

Strategy (data-parallel, 2 batches/core, 8 cores; F=512-pixel tiles):
  * fx runs channel-major ([97,2,F] moving, fp16) with the 3 gate channels
    folded into the stationary (M=99 piece) -- 4 matmuls.
  * Gates are transposed on the PE ([3,128] -> [128,3] per 128-px group) so
    the gating multiply becomes DVE tensor_scalar with per-partition fp32
    scalars (4x perf mode on fp16 SBUF operands): 12 ts + 2 adds per tile.
  * ctx is transposed back to channel-major on the PE (8 tiny transposes)
    and copied PSUM->SBUF; mod is 4 channel-major matmuls; xo=(mod+hb)*q is
    scalar_tensor_tensor on Pool.
  * proj uses xo as the *stationary* operand ([96,128] chunks) with the
    weight matrix moving, so y lands pixel-major [128pix,192ch] in PSUM
    (8 matmuls of 192 rows + 4 bias-broadcast matmuls) -- the LayerNorm
    tail then needs no partition reductions: var = Act Square+accum_out,
    istd is a per-partition scalar, apply = stt((y*istd) + x0') with the
    residual and ln bias folded into a host-precomputed pixel-major x0'.
  * All activation tensors fp16 (DMA bytes halved); matmuls fp16 (1 cyc/row).
"""

import sys

sys.path.insert(0, "/opt/trn_rl_repo")

import numpy as np

import bass_rust
import concourse.bass as bass
import concourse.mybir as mybir
import concourse.tile as tile
from concourse.bass_utils import run_bass_kernel_spmd
from concourse.vector_clock import ScopedClock

# ---------------------------------------------------------------------------
# Workaround: this walrus build accepts only one sem wait per instruction
# ("Too many sync wait commands"). (1) chain the Tile tail drain's waits;
# (2) post-pass that moves excess waits onto NoOps inserted just before the
# offending instruction on the same engine.


def _patched_drain_and_barrier(self, tick_clock, wait_clock):
    nc = self.nc
    drain_inst = nc.sync.drain()
    wait_clock.add_sem_waits(
        drain_inst.ins, ScopedClock({None: tick_clock.global_clock})
    )
    si = drain_inst.ins.sync_info
    if si is not None and len(si.on_wait) > 1:
        waits = list(si.on_wait)
        drain_inst.ins.sync_info = bass_rust.SyncInfo(
            on_wait=waits[:1], on_update=list(si.on_update)
        )
        for w in waits[1:]:
            d2 = nc.sync.drain()
            d2.ins.sync_info = bass_rust.SyncInfo(on_wait=[w], on_update=[])
    nc.all_engine_barrier()
    assert self.sems is not None
    popped = nc._tile_sem_poison_stack.pop()
    assert popped is self._sem_poison
    nc.clear_and_free_semaphores(list(self.sems.allocated().values()))
    nc.all_engine_barrier()


tile.TileContext._drain_and_barrier = _patched_drain_and_barrier

_WAIT_LIMIT = 1


def _split_excess_waits(nc):
    k = 0
    for f in nc.m.functions:
        for b in f.blocks:
            il = b.instructions
            new = []
            for inst in il:
                si = inst.sync_info
                if si is not None and len(si.on_wait) > _WAIT_LIMIT:
                    waits = list(si.on_wait)
                    excess, keep = waits[:-_WAIT_LIMIT], waits[-_WAIT_LIMIT:]
                    for w in excess:
                        nop = mybir.InstNoOp(name=f"wsplit-{k}",
                                             engine=inst.engine)
                        nop.sync_info = bass_rust.SyncInfo(on_wait=[w],
                                                           on_update=[])
                        new.append(nop)
                        k += 1
                    inst.sync_info = bass_rust.SyncInfo(
                        on_wait=keep, on_update=list(si.on_update))
                new.append(inst)
            il[:] = new
    return k
# ---------------------------------------------------------------------------

FP32 = mybir.dt.float32
F16 = mybir.dt.float16
AF = mybir.ActivationFunctionType
OP = mybir.AluOpType

NCORES = 8
N_FULL, C, H, W, L = 16, 192, 128, 128, 3
HW = H * W
NS = N_FULL // NCORES          # batch per core
MAC = 1024                     # pixels per DMA macro-tile
F = 512                        # pixels per inner tile
NMAC = HW // MAC
NF = MAC // F
NCH = F // 256                 # 256-px chunks per F tile (=2)
EPS = 1e-6
RSC = float(1.0 / np.sqrt(C))  # variance via Square(in*RSC) accumulation

_prog_cache = {}


def _ileave(base):
    """[P, 256] contiguous AP -> [P, (par 2)(idx 128)] interleaved view.

    Free enumeration (par, idx) maps position par*128+idx to element
    par + 2*idx, so channel-major pixel order within each 256-px chunk
    matches the pixel-major packing (partition p <-> pixel 2p+par).
    """
    return bass_rust.AP(tensor=base.tensor, offset=base.offset,
                        ap=[base.ap[0], [1, 2], [2, 128]])


def _build_program(general_lnw, general_pb):
    nc = bass.Bass(trn_type="TRN2")

    d_x = nc.dram_tensor("x", [NS, 194, HW], F16, kind="ExternalInput")
    d_xl = [nc.dram_tensor(f"xl{l}", [NS, HW, C], F16, kind="ExternalInput")
            for l in range(L)]
    d_x0 = nc.dram_tensor("x0p", [NS, HW, C], F16, kind="ExternalInput")
    d_fw = [nc.dram_tensor(f"fw{j}", [97, 195], F16, kind="ExternalInput")
            for j in range(2)]
    d_hw = [nc.dram_tensor(f"hw{j}", [96, 192], F16, kind="ExternalInput")
            for j in range(2)]
    d_pj = [nc.dram_tensor(f"pj{j}", [96, 192], F16, kind="ExternalInput")
            for j in range(2)]
    d_pbt = nc.dram_tensor("pbt", [1, 192], F16, kind="ExternalInput")
    d_hb = nc.dram_tensor("hb", [96, 2], FP32, kind="ExternalInput")
    d_id16 = nc.dram_tensor("id16", [128, 128], F16, kind="ExternalInput")
    d_id32 = nc.dram_tensor("id32", [99, 3], F16, kind="ExternalInput")
    d_one1 = nc.dram_tensor("one1", [1, 128], F16, kind="ExternalInput")
    d_eps = nc.dram_tensor("epsc", [128, 1], FP32, kind="ExternalInput")
    if general_lnw:
        d_lwp = nc.dram_tensor("lwp", [128, 192], F16, kind="ExternalInput")
    d_out = nc.dram_tensor("out", [NS, HW, C], F16, kind="ExternalOutput")

    # x: [NS, 194, HW] -> [NS][97, 2, HW]; aug channel a = 97*j + p
    vx = d_x[:, :, :].rearrange("n (j p) w -> n p j w", j=2)
    # pixel-major tensors: pixel = 1024*m + 256*cc + 2*p + par
    vxl = [t[:, :, :].rearrange("n (m cc p par) c -> n m p cc (par c)",
                                m=NMAC, cc=4, p=128, par=2) for t in d_xl]
    vx0 = d_x0[:, :, :].rearrange("n (m cc p par) c -> n m p cc (par c)",
                                  m=NMAC, cc=4, p=128, par=2)
    vout = d_out[:, :, :].rearrange("n (m cc p par) c -> n m p cc (par c)",
                                    m=NMAC, cc=4, p=128, par=2)

    from contextlib import ExitStack
    with tile.TileContext(nc) as tc, ExitStack() as ctx:
        sing = ctx.enter_context(tc.tile_pool(name="sing", bufs=1))
        inp = ctx.enter_context(tc.tile_pool(name="inp", bufs=4))
        qgp = ctx.enter_context(tc.tile_pool(name="qgp", bufs=4))
        gat = ctx.enter_context(tc.tile_pool(name="gat", bufs=2))
        csb = ctx.enter_context(tc.tile_pool(name="csb", bufs=2))
        xop = ctx.enter_context(tc.tile_pool(name="xop", bufs=3))
        tlp = ctx.enter_context(tc.tile_pool(name="tlp", bufs=2))
        zp = ctx.enter_context(tc.tile_pool(name="zp", bufs=3))
        pfx = ctx.enter_context(tc.tile_pool(name="pfx", bufs=1, space="PSUM"))
        pgp = ctx.enter_context(tc.tile_pool(name="pgp", bufs=1, space="PSUM"))
        pcx = ctx.enter_context(tc.tile_pool(name="pcx", bufs=1, space="PSUM"))
        pmd = ctx.enter_context(tc.tile_pool(name="pmd", bufs=1, space="PSUM"))
        ppj = ctx.enter_context(tc.tile_pool(name="ppj", bufs=2, space="PSUM"))

        # ---- load constants / weights (once) ----
        def loadw(dram, shape, dt, tag):
            t = sing.tile(shape, dt, tag=tag, name=tag)
            nc.sync.dma_start(out=t,
                              in_=dram[tuple(slice(0, s) for s in shape)])
            return t

        fw = [loadw(d_fw[j], [97, 195], F16, f"fw{j}") for j in range(2)]
        hwt = [loadw(d_hw[j], [96, 192], F16, f"hw{j}") for j in range(2)]
        pjc = [loadw(d_pj[j], [96, 192], F16, f"pj{j}") for j in range(2)]
        pbt = loadw(d_pbt, [1, 192], F16, "pbt")
        hb = loadw(d_hb, [96, 2], FP32, "hb")
        id16 = loadw(d_id16, [128, 128], F16, "id16")
        id32 = loadw(d_id32, [99, 3], F16, "id32")
        one1 = loadw(d_one1, [1, 128], F16, "one1")
        eps_t = loadw(d_eps, [128, 1], FP32, "epsc")
        lwp = loadw(d_lwp, [128, 192], F16, "lwp") if general_lnw else None

        mm = nc.tensor.matmul
        tiles = [(n, im, fi) for n in range(NS) for im in range(NMAC)
                 for fi in range(NF)]
        T = len(tiles)
        macs = {}
        st = {}

        def load_macro(n, im):
            o0 = im * MAC
            x_t = inp.tile([97, 2, MAC], F16, tag="x", name="x_t")
            nc.sync.dma_start(out=x_t, in_=vx[n, :, :, o0:o0 + MAC])
            xl_t = []
            for l in range(L):
                t = inp.tile([128, 4, 2 * C], F16, tag=f"xl{l}",
                             name=f"xl{l}")
                nc.sync.dma_start(out=t, in_=vxl[l][n, im])
                xl_t.append(t)
            x0_t = inp.tile([128, 4, 2 * C], F16, tag="x0", name="x0_t")
            nc.sync.dma_start(out=x0_t, in_=vx0[n, im])
            macs[(n, im)] = (x_t, xl_t, x0_t)

        # ---- stage S0: DMA + fx matmuls + q/gates PSUM->SBUF copies ----
        def _xmov(x_t, j, fi):
            """fx moving operand: [97, F] with free order (cc, par, idx)
            matching the pixel-major group packing."""
            base = x_t[:, j, fi * F:(fi + 1) * F]
            return bass_rust.AP(tensor=base.tensor, offset=base.offset,
                                ap=[base.ap[0], [256, 2], [1, 2], [2, 128]])

        def s0(i):
            n, im, fi = tiles[i]
            if (n, im) not in macs:
                load_macro(n, im)
            x_t, xl_t, x0_t = macs[(n, im)]
            fx = pfx.tile([99, 2, F], FP32, tag="fx", name="fx")
            for j in range(2):
                mm(fx[:, 0, :], fw[j][:, 0:99], _xmov(x_t, j, fi),
                   start=(j == 0), stop=(j == 1))
            for j in range(2):
                mm(fx[0:96, 1, :], fw[j][:, 99:195], _xmov(x_t, j, fi),
                   start=(j == 0), stop=(j == 1))
            # q + gates to SBUF in one copy (fp32: gate transposes/ts
            # scalars need fp32)
            qg = qgp.tile([99, 2, F], F16, tag="qg", name="qg")
            nc.scalar.activation(qg, fx, AF.Copy)
            st[i] = {"n": n, "im": im, "fi": fi,
                     "xl_t": xl_t, "x0_t": x0_t, "qg": qg}

        # ---- stage S1: gate transposes + gating (pixel-major) ----
        def s1(i):
            d = st[i]
            fi = d["fi"]
            qg0 = d["qg"][:, 0, :]
            xl_t = d["xl_t"]
            gts = gat.tile([3, F], F16, tag="gts", name="gts")
            nc.sync.dma_start(out=gts, in_=qg0[96:99, :])
            gps = pgp.tile([128, 4, 4], F16, tag="gps", name="gps")
            for cp in range(4):
                nc.tensor.transpose(gps[:, cp, 0:3],
                                    gts[:, cp * 128:(cp + 1) * 128],
                                    id32[0:3, :])
            gpsf = gat.tile([128, 4, 4], FP32, tag="gpsf", name="gpsf")
            nc.vector.tensor_scalar(gpsf, gps, 1.0, None, OP.mult)
            t0 = gat.tile([128, 4, C], F16, tag="t0", name="t0")
            t1 = gat.tile([128, 4, C], F16, tag="t1", name="t1")
            t2 = gat.tile([128, 4, C], F16, tag="t2", name="t2")
            tl = [t0, t1, t2]
            for l in range(L):
                for cc in range(NCH):
                    for par in range(2):
                        cp = 2 * cc + par
                        eng = nc.vector if l < 2 else nc.gpsimd
                        eng.tensor_scalar(
                            tl[l][:, cp, :],
                            xl_t[l][:, 2 * fi + cc,
                                    par * C:(par + 1) * C],
                            gpsf[:, cp, l:l + 1], None, OP.mult)
            cxa = gat.tile([128, 4, C], F16, tag="cxa", name="cxa")
            nc.vector.tensor_tensor(cxa, t0, t1, OP.add)
            cxb = gat.tile([128, 4, C], F16, tag="cxb", name="cxb")
            nc.vector.tensor_tensor(cxb, cxa, t2, OP.add)
            d["ctx_px"] = cxb

        # ---- stage S2: ctx transposes back to channel-major + copy ----
        def s2(i):
            d = st[i]
            cxb = d["ctx_px"]
            ccm = pcx.tile([96, 2, F], F16, tag="ccm", name="ccm")
            for j in range(2):
                for cp in range(4):
                    nc.tensor.transpose(
                        ccm[:, j, cp * 128:(cp + 1) * 128],
                        cxb[:, cp, 96 * j:96 * (j + 1)], id16)
            csb_t = csb.tile([96, 2, F], F16, tag="csb", name="csb_t")
            nc.vector.tensor_scalar(csb_t, ccm, 1.0, None, OP.mult)
            d["ctx_sb"] = csb_t

        # ---- stage S3: mod matmuls + xo = (mod+hb)*q on Pool ----
        def s3(i):
            d = st[i]
            ctx_sb = d["ctx_sb"]
            mod = pmd.tile([96, 2, F], FP32, tag="mod", name="mod")
            for m in range(2):
                for j in range(2):
                    mm(mod[:, m, :], hwt[j][:, 96 * m:96 * (m + 1)],
                       ctx_sb[:, j, :], start=(j == 0), stop=(j == 1))
            msb = xop.tile([96, 2, F], F16, tag="msb", name="msb")
            for m in range(2):
                nc.scalar.activation(msb[:, m, :], mod[:, m, :],
                                     AF.Identity, bias=hb[:, m:m + 1])
            xo = xop.tile([96, 2, F], F16, tag="xo", name="xo")
            nc.vector.tensor_tensor(xo, msb, d["qg"][0:96, :, :], OP.mult)
            d["xo"] = xo

        # ---- stage S4a/S4b: proj (pixel-major out) + LN tail, one half
        # (= one 256-px chunk pair... 2 of the 4 groups) per step so the
        # pj PSUM pool double-buffers within the bank budget ----
        zmacs = {}

        def s4h(i, h):
            d = st[i]
            xo = d["xo"]
            x0_t = d["x0_t"]
            n, im, fi = d["n"], d["im"], d["fi"]
            if (n, im) not in zmacs:
                zmacs[(n, im)] = zp.tile([128, 4, 2 * C], F16, tag="zmac",
                                         name="zmac")
            zmac = zmacs[(n, im)]
            pjh = ppj.tile([128, 2, 192], FP32, tag="pjh", name="pjh")
            for ci in range(2):
                cp = 2 * h + ci
                out = pjh[:, ci, :]
                for j in range(2):
                    mm(out, xo[:, j, cp * 128:(cp + 1) * 128], pjc[j],
                       start=(j == 0), stop=(j == 1 and not general_pb))
                if general_pb:
                    mm(out, one1, pbt, start=False, stop=True)
            var2 = tlp.tile([128, 2], FP32, tag="var2", name="var2")
            sqs = tlp.tile([128, 2, 192], F16, tag="sqs", name="sqs")
            nc.scalar.activation(sqs, pjh, AF.Square, scale=RSC)
            nc.vector.tensor_reduce(var2, sqs,
                                    mybir.AxisListType.X, OP.add)
            lnv = tlp.tile([128, 2], FP32, tag="lnv", name="lnv")
            nc.scalar.activation(lnv, var2, AF.Ln, bias=eps_t)
            istd = tlp.tile([128, 2], FP32, tag="istd", name="istd")
            nc.scalar.activation(istd, lnv, AF.Exp, scale=-0.5)
            z1 = tlp.tile([128, 2 * C], F16, tag="z1", name="z1")
            nc.scalar.activation(z1[:, 0:C], pjh[:, 0, :], AF.Identity,
                                 scale=istd[:, 0:1])
            nc.vector.tensor_scalar(z1[:, C:2 * C], pjh[:, 1, :],
                                    istd[:, 1:2], None, OP.mult)
            if general_lnw:
                for ci in range(2):
                    nc.vector.tensor_tensor(z1[:, ci * C:(ci + 1) * C],
                                            z1[:, ci * C:(ci + 1) * C],
                                            lwp, OP.mult)
            nc.vector.tensor_tensor(zmac[:, 2 * fi + h, :], z1,
                                    x0_t[:, 2 * fi + h, :], OP.add)
            if h == 1:
                st.pop(i)
                if fi == NF - 1:
                    nc.sync.dma_start(out=vout[n, im], in_=zmac)
                    del zmacs[(n, im)]
                    del macs[(n, im)]

        # ---- software pipeline: 6 slots ----
        import os
        order = os.environ.get("K5_ORDER", "134502")
        slots = {"0": (s0, 0), "1": (s1, 1), "2": (s2, 2), "3": (s3, 3),
                 "4": (s4h, 4), "5": (s4h, 5)}
        seq = []
        for ch in order:
            if ch == "0":
                seq.append(("s0", 0))
            elif ch == "1":
                seq.append(("s1", 1))
            elif ch == "2":
                seq.append(("s2", 2))
            elif ch == "3":
                seq.append(("s3", 3))
            elif ch == "4":
                seq.append(("s4a", 4))
            elif ch == "5":
                seq.append(("s4b", 5))
        fns = {"s0": s0, "s1": s1, "s2": s2, "s3": s3,
               "s4a": lambda i: s4h(i, 0), "s4b": lambda i: s4h(i, 1)}
        for k in range(T + 5):
            for name, lag in seq:
                i = k - lag
                if 0 <= i < T:
                    fns[name](i)

    return nc


def _get_program(general_lnw=False, general_pb=False):
    key = ("nc", general_lnw, general_pb)
    if key not in _prog_cache:
        nc = _build_program(general_lnw, general_pb)
        _split_excess_waits(nc)
        _prog_cache[key] = nc
    return _prog_cache[key]


def kernel(**inputs):
    x = np.ascontiguousarray(inputs["x"], dtype=np.float32)
    x_list = np.ascontiguousarray(inputs["x_list"], dtype=np.float32)
    f_w = np.asarray(inputs["f_w"], dtype=np.float32)
    f_b = np.asarray(inputs["f_b"], dtype=np.float32)
    h_w = np.asarray(inputs["h_w"], dtype=np.float32)
    h_b = np.asarray(inputs["h_b"], dtype=np.float32)
    proj_w = np.asarray(inputs["proj_w"], dtype=np.float32)
    proj_b = np.asarray(inputs["proj_b"], dtype=np.float32)
    ln_w = np.asarray(inputs["ln_w"], dtype=np.float32)
    ln_b = np.asarray(inputs["ln_b"], dtype=np.float32)

    general_lnw = not np.allclose(ln_w, 1.0)
    general_pb = not np.allclose(proj_b, 0.0)

    # ---- host-side weight prep (tiny) ----
    # fx stationary [97, 195] per j; col order: q outs 0..95, gates, q outs
    # 96..191 (so the M-split 0:99 / 99:195 keeps slices contiguous).
    fwj = []
    for j in range(2):
        a = np.zeros((97, 195), dtype=np.float32)
        blk = f_w[:, 96 * j:96 * (j + 1)]           # [195 outs, 96 ins_j]
        a[0:96, 0:96] = blk[0:96].T
        a[0:96, 96:99] = blk[192:195].T
        a[0:96, 99:195] = blk[96:192].T
        if j == 0:
            a[96, 0:96] = f_b[0:96]
            a[96, 96:99] = f_b[192:195]
            a[96, 99:195] = f_b[96:192]
        fwj.append(a.astype(np.float16))
    hwj = [np.ascontiguousarray(h_w[:, 96 * j:96 * (j + 1)].T).astype(
        np.float16) for j in range(2)]
    w_mu = proj_w.mean(axis=0)
    pw = proj_w - w_mu[None, :]                     # mean-folded [out, in]
    pjj = [np.ascontiguousarray(pw[:, 96 * j:96 * (j + 1)].T).astype(
        np.float16) for j in range(2)]
    pbt = (proj_b - proj_b.mean())[None, :].astype(np.float16)
    hbv = np.ascontiguousarray(h_b.reshape(2, 96).T).astype(np.float32)

    # ---- host-side input prep ----
    xs = x.reshape(NCORES, NS, C, HW)
    xa = np.empty((NCORES, NS, 194, HW), dtype=np.float16)
    xa[:, :, 0:96] = xs[:, :, 0:96]
    xa[:, :, 96] = 1.0
    xa[:, :, 97:193] = xs[:, :, 96:192]
    xa[:, :, 193] = 1.0
    xls = np.ascontiguousarray(
        x_list.reshape(L, NCORES, NS, C, HW).transpose(0, 1, 2, 4, 3)
    ).astype(np.float16)                            # [L, NC, NS, HW, C]
    x0p = xs.transpose(0, 1, 3, 2) + ln_b[None, None, None, :]
    x0p = np.ascontiguousarray(x0p).astype(np.float16)  # [NC, NS, HW, C]

    common = {
        "fw0": fwj[0], "fw1": fwj[1],
        "hw0": hwj[0], "hw1": hwj[1],
        "pj0": pjj[0], "pj1": pjj[1],
        "pbt": pbt, "hb": hbv,
        "id16": np.eye(128, dtype=np.float16),
        "id32": np.concatenate([np.eye(3, dtype=np.float16),
                                np.zeros((96, 3), np.float16)], axis=0),
        "one1": np.ones((1, 128), dtype=np.float16),
        "epsc": np.full((128, 1), EPS, dtype=np.float32),
    }
    if general_lnw:
        common["lwp"] = np.ascontiguousarray(
            np.broadcast_to(ln_w[None, :], (128, 192))).astype(np.float16)
    in_maps = []
    for c in range(NCORES):
        m = dict(common)
        m["x"] = xa[c]
        m["x0p"] = x0p[c]
        for l in range(L):
            m[f"xl{l}"] = xls[l, c]
        in_maps.append(m)

    nc = _get_program(general_lnw, general_pb)
    res = run_bass_kernel_spmd(nc, in_maps, core_ids=list(range(NCORES)))
    out = np.stack([r["out"] for r in res.results], axis=0)  # [NC,NS,HW,C]
    out = out.astype(np.float32).transpose(0, 1, 3, 2)       # [NC,NS,C,HW]
    return np.ascontiguousarray(out.reshape(N_FULL, C, H, W))
